# revision 1
# baseline (speedup 1.0000x reference)
"""Multi-head attention (B=2, S=2048, D=1024, H=16) on 8 TRN2 NeuronCores.

Sharding: data-parallel over batch (2 groups of 4 cores) x head-parallel
(4 heads per core). W_q/W_k/W_v are column-sharded by head, W_o is
row-sharded; the 4 partial W_o outputs per batch are summed on the host
(the unshard step), which also undoes the device-side transposed layout.

Per-core kernel design: projection inputs (X, W_q/k/v) stream as bf16
(halves the HBM traffic); everything downstream - scores, probs, V, W_o -
is fp32-in-memory with float32r matmul inputs, which runs the PE at full
rate with ~1.5e-4 matmul error. End-to-end relative error ~4e-3.

  - Host pre-transposes X (Q/K/V inputs) and the weight slices so that
    every matmul contraction sits on the partition dim.
  - q/k projections produce qT/kT in [128 = 2 heads x 64 d, S] layout;
    the 1/sqrt(d_k) scale is folded into W_q/b_q on the host.
  - v projection produces v in natural [S, d] layout with a ones column
    appended per head, so the P@V matmul accumulates the softmax
    denominator (row 64 of the accumulator) for free.
  - scores are computed transposed ([k, sq] blocks); softmax skips the
    max-subtraction (scores are O(5) here, exp is safe in fp32), the
    denominator reciprocal is broadcast across partitions with a rank-1
    PE outer product.
  - causal structure: fully-masked [128 k x 512 sq] blocks are skipped,
    diagonal blocks are zeroed post-exp with gpsimd.affine_select. The
    block plan is derived from the actual mask input at call time, with
    a dense additive-mask fallback for non-causal patterns.
"""

import os

import numpy as np

_B, _S, _D, _H, _DK = 2, 2048, 1024, 16, 64
_HPC = 4          # heads per core
_NCORES = 8
_CPG = 4          # cores per (batch) group
_DPC = _HPC * _DK # 256 projection dims per core
_NEG = -1e9

_program_cache = {}
LAST_RESULTS = None  # BassKernelResults of the most recent run (for profiling)


def _analyze_mask(mask):
    """Classify each [128 k, 512 sq] block of mask^T. Returns (plan, dense).

    plan[i] = tuple of (j, mode, param) for sq-tile i; mode 0 = no mask,
    1 = causal affine_select (param = base), 2 = dense additive mask
    (param = index into dense blocks). Fully-masked blocks are omitted.
    """
    maskT = np.ascontiguousarray(mask.T)
    plan = []
    dense = []
    p_idx = np.arange(128)[:, None]
    s_idx = np.arange(512)[None, :]
    for i in range(_S // 512):
        row = []
        for j in range(_S // 128):
            blk = maskT[j * 128:(j + 1) * 128, i * 512:(i + 1) * 512]
            nz = blk != 0.0
            if nz.all():
                continue  # fully masked: block contributes nothing
            if not nz.any():
                row.append((j, 0, 0))
                continue
            base = i * 512 - j * 128
            causal = (s_idx + i * 512) < (p_idx + j * 128)
            if np.array_equal(nz, causal) and np.all(blk[nz] == 1.0):
                row.append((j, 1, base))
            else:
                row.append((j, 2, len(dense)))
                dense.append(blk * np.float32(_NEG))
        plan.append(tuple(row))
    if dense:
        dense_np = np.stack(dense).astype(np.float32)
    else:
        dense_np = np.zeros((1, 128, 512), np.float32)
    return tuple(plan), dense_np


def _build_program(plan, nblk):
    import concourse.bass as bass  # noqa: F401  (registers engine classes)
    import concourse.tile as tile
    from concourse import bacc, mybir

    F32 = mybir.dt.float32
    F32R = mybir.dt.float32r
    BF16 = mybir.dt.bfloat16
    AF = mybir.ActivationFunctionType
    ALU = mybir.AluOpType
    ts = bass.ts

    nc = bacc.Bacc(None, target_bir_lowering=False, debug=False)

    xq = nc.dram_tensor("xq", [_D, _S], BF16, kind="ExternalInput").ap()
    xk = nc.dram_tensor("xk", [_D, _S], BF16, kind="ExternalInput").ap()
    xv = nc.dram_tensor("xv", [_D, _S], BF16, kind="ExternalInput").ap()
    wq = nc.dram_tensor("wq", [_D, _DPC], BF16, kind="ExternalInput").ap()
    wk = nc.dram_tensor("wk", [_D, _DPC], BF16, kind="ExternalInput").ap()
    wv = nc.dram_tensor("wv", [_D, _DPC], BF16, kind="ExternalInput").ap()
    wo = nc.dram_tensor("wo", [_DPC, _D], F32R, kind="ExternalInput").ap()
    bq = nc.dram_tensor("bq", [_DPC], F32, kind="ExternalInput").ap()
    bk = nc.dram_tensor("bk", [_DPC], F32, kind="ExternalInput").ap()
    bvb = nc.dram_tensor("bvb", [128, _DPC], F32, kind="ExternalInput").ap()
    mblk = nc.dram_tensor("mblk", [nblk, 128, 512], F32, kind="ExternalInput").ap()
    y = nc.dram_tensor("y", [_D, _S], F32, kind="ExternalOutput").ap()

    with tile.TileContext(nc) as tc:
        from contextlib import ExitStack
        with ExitStack() as ctx:
            wpool = ctx.enter_context(tc.tile_pool(name="w", bufs=1))
            cpool = ctx.enter_context(tc.tile_pool(name="const", bufs=1))
            xcol_bufs = 6
            if any(m == 2 for row in plan for (_, m, _) in row) and nblk > 2:
                xcol_bufs = 5  # reclaim SBUF for the streamed mask tiles
            xpool = ctx.enter_context(tc.tile_pool(name="xcol", bufs=xcol_bufs))
            biga = ctx.enter_context(tc.tile_pool(name="biga", bufs=1))
            probp = ctx.enter_context(tc.tile_pool(name="probs", bufs=6))
            bcp = ctx.enter_context(tc.tile_pool(name="bc", bufs=6))
            recp = ctx.enter_context(tc.tile_pool(name="rec", bufs=4))
            yp = ctx.enter_context(tc.tile_pool(name="y", bufs=4))
            has_dense = any(m == 2 for row in plan for (_, m, _) in row)
            resident_mask = has_dense and nblk <= 2
            need_stream = has_dense and not resident_mask
            mpool = (
                ctx.enter_context(tc.tile_pool(name="mstream", bufs=3))
                if need_stream else None
            )
            mmps = ctx.enter_context(tc.tile_pool(name="mmps", bufs=2, space="PSUM"))
            spsp = ctx.enter_context(tc.tile_pool(name="sps", bufs=2, space="PSUM"))
            accp = ctx.enter_context(tc.tile_pool(name="acc", bufs=2, space="PSUM"))

            xq_r = xq.rearrange("(m p) s -> p m s", p=128)
            xk_r = xk.rearrange("(m p) s -> p m s", p=128)
            xv_r = xv.rearrange("(m p) s -> p m s", p=128)

            def dma_m2(out_tile, in_ap):
                # split the m (dim-1) axis into halves so dependents on the
                # first m-chunks unblock at half the transfer
                nc.sync.dma_start(out=out_tile[:, 0:4, :], in_=in_ap[:, 0:4, :])
                nc.sync.dma_start(out=out_tile[:, 4:8, :], in_=in_ap[:, 4:8, :])

            # --- critical-path DMAs first: the first sq column's x plus
            # the q/k weights, interleaved by m-halves so the projection
            # m-loops start as early as possible
            first_st = 0
            xq_t = xpool.tile([128, 8, 512], BF16, tag="xcol", name="xq_tc0")
            wq_sb = wpool.tile([128, 8, _DPC], BF16, tag="wq")
            xk_t = xpool.tile([128, 8, 512], BF16, tag="xcol", name="xk_tc0")
            wk_sb = wpool.tile([128, 8, _DPC], BF16, tag="wk")
            wv_sb = wpool.tile([128, 8, _DPC], BF16, tag="wv")
            wq_r = wq.rearrange("(m p) d -> p m d", p=128)
            wk_r = wk.rearrange("(m p) d -> p m d", p=128)
            wv_r = wv.rearrange("(m p) d -> p m d", p=128)
            for lo, hi in ((0, 4), (4, 8)):
                nc.sync.dma_start(out=xq_t[:, lo:hi, :],
                                  in_=xq_r[:, lo:hi, ts(first_st, 512)])
                nc.sync.dma_start(out=wq_sb[:, lo:hi, :], in_=wq_r[:, lo:hi, :])
                nc.sync.dma_start(out=xk_t[:, lo:hi, :],
                                  in_=xk_r[:, lo:hi, ts(first_st, 512)])
                nc.sync.dma_start(out=wk_sb[:, lo:hi, :], in_=wk_r[:, lo:hi, :])
            dma_m2(wv_sb, wv_r)

            bq_sb = cpool.tile([128, 2], F32, tag="bq")
            nc.sync.dma_start(out=bq_sb, in_=bq.rearrange("(h p) -> p h", p=128))
            bk_sb = cpool.tile([128, 2], F32, tag="bk")
            nc.sync.dma_start(out=bk_sb, in_=bk.rearrange("(h p) -> p h", p=128))
            bvb_sb = cpool.tile([128, _DPC], F32, tag="bvb")
            nc.sync.dma_start(out=bvb_sb, in_=bvb)
            if resident_mask:
                mask_sb = cpool.tile([128, nblk, 512], F32, tag="mask")
                nc.sync.dma_start(
                    out=mask_sb, in_=mblk.rearrange("n p s -> p n s")
                )
            wo_sb = wpool.tile([128, 2, _D], F32R, tag="wo")
            nc.sync.dma_start(out=wo_sb, in_=wo.rearrange("(c p) o -> p c o", p=128))

            aff_params = sorted({p for row in plan for (_, m, p) in row
                                 if m == 1})
            use_m01 = 0 < len(aff_params) <= 4
            if use_m01:
                m01 = cpool.tile([128, len(aff_params), 512], F32, tag="m01")
                nc.vector.memset(m01, 1.0)
                for oi, bp in enumerate(aff_params):
                    nc.gpsimd.affine_select(
                        out=m01[:, oi, :], in_=m01[:, oi, :],
                        compare_op=ALU.is_ge, fill=0.0, base=bp,
                        channel_multiplier=-1, pattern=[[1, 512]],
                    )

            ones32 = cpool.tile([1, 64], F32, tag="ones32")
            nc.vector.memset(ones32, 1.0)
            ones_r = cpool.tile([1, 64], F32R, tag="ones_r")
            nc.vector.tensor_copy(ones_r, ones32)
            onecol = cpool.tile([128, 1], F32, tag="onecol")
            nc.vector.memset(onecol, 1.0)

            # --- big SBUF state ---
            qT = biga.tile([128, 2, _S], F32R, tag="qT")
            kT = biga.tile([128, 2, _S], F32R, tag="kT")
            vsb = biga.tile([128, 16, _HPC * 65], F32R, tag="v")
            attn = biga.tile([128, 2, _S], F32R, tag="attn")

            # ones columns of v (softmax denominator trick)
            for sc in range(16):
                for h in range(_HPC):
                    nc.vector.tensor_copy(
                        vsb[:, sc, h * 65 + 64:h * 65 + 65], onecol
                    )

            # v-projection emitted lazily per 512-wide k-column group, the
            # first time any PV needs a chunk from it
            v_pending = set(range(4))

            def ensure_vgroup(col):
                if col not in v_pending:
                    return
                v_pending.discard(col)
                xv_t = xpool.tile([128, 8, 512], BF16, tag="xcol",
                                  name=f"xv_t{col}")
                dma_m2(xv_t, xv_r[:, :, ts(col, 512)])
                for c in range(4):
                    vps = mmps.tile([128, 512], F32, tag="mm", name="vps")
                    for m in range(8):
                        nc.tensor.matmul(
                            vps[:, 0:_DPC], lhsT=xv_t[:, m, ts(c, 128)],
                            rhs=wv_sb[:, m, :], start=(m == 0), stop=(m == 7),
                        )
                    sc = col * 4 + c
                    nc.vector.tensor_add(
                        vsb[:, sc, 0:260].rearrange(
                            "p (h x) -> p h x", x=65)[:, :, 0:64],
                        vps[:, 0:_DPC].rearrange("p (h x) -> p h x", x=64),
                        bvb_sb.rearrange("p (h x) -> p h x", x=64),
                    )

            # --- fused pipeline over sq columns (ascending: attention at
            # column i needs kT/v for all k-chunks <= i)
            def emit_outproj(st):
                # output projection for sq column st (row-sharded partial)
                for oc in range(8):
                    yps = accp.tile([128, 512], F32, tag="acc", name="yps")
                    for cc in range(2):
                        nc.tensor.matmul(
                            yps, lhsT=wo_sb[:, cc, ts(oc, 128)],
                            rhs=attn[:, cc, ts(st, 512)],
                            start=(cc == 0), stop=(cc == 1),
                        )
                    y_sb = yp.tile([128, 512], F32, tag="y", name="y_sb")
                    nc.vector.tensor_copy(y_sb, yps)
                    nc.sync.dma_start(
                        out=y[oc * 128:(oc + 1) * 128, ts(st, 512)], in_=y_sb
                    )

            for idx, st in enumerate((0, 1, 2, 3)):
                if idx > 0:
                    xq_t = xpool.tile([128, 8, 512], BF16, tag="xcol",
                                      name=f"xq_t{st}")
                    dma_m2(xq_t, xq_r[:, :, ts(st, 512)])
                    xk_t = xpool.tile([128, 8, 512], BF16, tag="xcol",
                                      name=f"xk_t{st}")
                    dma_m2(xk_t, xk_r[:, :, ts(st, 512)])

                # q/k projections for this column of sq
                for dh in range(2):
                    qps = mmps.tile([128, 512], F32, tag="mm", name="qps")
                    for m in range(8):
                        nc.tensor.matmul(
                            qps, lhsT=wq_sb[:, m, ts(dh, 128)], rhs=xq_t[:, m, :],
                            start=(m == 0), stop=(m == 7),
                        )
                    nc.vector.tensor_scalar(
                        qT[:, dh, ts(st, 512)], qps, bq_sb[:, dh:dh + 1], None,
                        ALU.add,
                    )
                    kps = mmps.tile([128, 512], F32, tag="mm", name="kps")
                    for m in range(8):
                        nc.tensor.matmul(
                            kps, lhsT=wk_sb[:, m, ts(dh, 128)], rhs=xk_t[:, m, :],
                            start=(m == 0), stop=(m == 7),
                        )
                    nc.vector.tensor_scalar(
                        kT[:, dh, ts(st, 512)], kps, bk_sb[:, dh:dh + 1], None,
                        ALU.add,
                    )

                # attention for sq tile i = st, both head pairs
                i = st
                blocks = plan[i]
                nj = len(blocks)
                for g in range(2):
                    acc = [
                        accp.tile([65, 512], F32, tag="acc", name=f"acc{g}{hh}")
                        for hh in range(2)
                    ]
                    for bi, (j, mode, param) in enumerate(blocks):
                        ensure_vgroup(j // 4)
                        sps = spsp.tile([128, 2, 512], F32, tag="sps", name="sps")
                        for hh in range(2):
                            nc.tensor.matmul(
                                sps[:, hh, :],
                                lhsT=kT[hh * 64:(hh + 1) * 64, g, ts(j, 128)],
                                rhs=qT[hh * 64:(hh + 1) * 64, g, ts(i, 512)],
                                start=True, stop=True,
                            )
                        if mode == 2:
                            if resident_mask:
                                mt = mask_sb[:, param, :]
                            else:
                                mt = mpool.tile([128, 512], F32, tag="mtile",
                                                name="mt")
                                nc.sync.dma_start(out=mt, in_=mblk[param])
                            for hh in range(2):
                                nc.vector.tensor_add(
                                    sps[:, hh, :], sps[:, hh, :], mt
                                )
                        probs = probp.tile([128, 2, 512], F32R, tag="probs",
                                           name="probs")
                        nc.scalar.activation(probs, sps, AF.Exp)
                        if mode == 1:
                            # masked cells satisfy s < p - base, p <= 127:
                            # only the first (128 - base) columns can be hit
                            ncols = min(512, 128 - param)
                            if ncols > 0 and use_m01:
                                oi = aff_params.index(param)
                                for hh in range(2):
                                    nc.vector.tensor_mul(
                                        probs[:, hh, 0:ncols],
                                        probs[:, hh, 0:ncols],
                                        m01[:, oi, 0:ncols],
                                    )
                            elif ncols > 0:
                                nc.gpsimd.affine_select(
                                    out=probs[:, :, 0:ncols],
                                    in_=probs[:, :, 0:ncols],
                                    compare_op=ALU.is_ge, fill=0.0,
                                    base=param, channel_multiplier=-1,
                                    pattern=[[0, 2], [1, ncols]],
                                )
                        for hh in range(2):
                            h = 2 * g + hh
                            nc.tensor.matmul(
                                acc[hh], lhsT=vsb[:, j, h * 65:(h + 1) * 65],
                                rhs=probs[:, hh, :],
                                start=(bi == 0), stop=(bi == nj - 1),
                            )
                    for hh in range(2):
                        rec = recp.tile([1, 512], F32R, tag="rec", name="rec")
                        with nc.allow_low_precision(
                            reason="softmax reciprocal; f32r storage"
                        ):
                            nc.vector.reciprocal(rec, acc[hh][64:65, :])
                        bc_ps = mmps.tile([64, 512], F32, tag="mm", name="bc_ps")
                        nc.tensor.matmul(bc_ps, lhsT=ones_r, rhs=rec)
                        bc_sb = bcp.tile([64, 512], F32, tag="bc", name="bc_sb")
                        nc.vector.tensor_copy(bc_sb, bc_ps)
                        nc.vector.tensor_mul(
                            attn[hh * 64:(hh + 1) * 64, g, ts(i, 512)],
                            acc[hh][0:64, :], bc_sb,
                        )

                emit_outproj(st)

    nc.compile()
    return nc


def kernel(**inputs):
    global LAST_RESULTS
    from concourse.bass_utils import run_bass_kernel_spmd

    Q = np.asarray(inputs["Q"], dtype=np.float32)
    K = np.asarray(inputs["K"], dtype=np.float32)
    V = np.asarray(inputs["V"], dtype=np.float32)
    mask = np.asarray(inputs["mask"], dtype=np.float32)
    Wq = np.asarray(inputs["Wq"], dtype=np.float32)
    bq = np.asarray(inputs["bq"], dtype=np.float32)
    Wk = np.asarray(inputs["Wk"], dtype=np.float32)
    bk = np.asarray(inputs["bk"], dtype=np.float32)
    Wv = np.asarray(inputs["Wv"], dtype=np.float32)
    bv = np.asarray(inputs["bv"], dtype=np.float32)
    Wo = np.asarray(inputs["Wo"], dtype=np.float32)
    bo = np.asarray(inputs["bo"], dtype=np.float32)

    plan, dense = _analyze_mask(mask)
    key = (plan, dense.shape[0])
    if key not in _program_cache:
        _program_cache[key] = _build_program(plan, dense.shape[0])
    nc = _program_cache[key]

    import ml_dtypes
    bf16 = ml_dtypes.bfloat16
    sc = np.float32(1.0 / np.sqrt(_DK))
    xqT = [np.ascontiguousarray(Q[b].T).astype(bf16) for b in range(_B)]
    xkT = [np.ascontiguousarray(K[b].T).astype(bf16) for b in range(_B)]
    xvT = [np.ascontiguousarray(V[b].T).astype(bf16) for b in range(_B)]

    in_maps = []
    for core in range(_NCORES):
        b = core // _CPG
        rows = slice((core % _CPG) * _DPC, (core % _CPG) * _DPC + _DPC)
        in_maps.append({
            "xq": xqT[b], "xk": xkT[b], "xv": xvT[b],
            "wq": np.ascontiguousarray((Wq[rows] * sc).T).astype(bf16),
            "wk": np.ascontiguousarray(Wk[rows].T).astype(bf16),
            "wv": np.ascontiguousarray(Wv[rows].T).astype(bf16),
            "wo": np.ascontiguousarray(Wo[:, rows].T),
            "bq": np.ascontiguousarray(bq[rows] * sc),
            "bk": np.ascontiguousarray(bk[rows]),
            "bvb": np.broadcast_to(bv[rows], (128, _DPC)).copy(),
            "mblk": dense,
        })

    trace = bool(int(os.environ.get("KERNEL_TRACE", "0")))
    LAST_RESULTS = run_bass_kernel_spmd(
        nc, in_maps, list(range(_NCORES)), trace=trace
    )

    out = np.empty((_B, _S, _D), np.float32)
    for b in range(_B):
        acc = np.zeros((_D, _S), np.float64)
        for c in range(_CPG):
            acc += LAST_RESULTS.results[b * _CPG + c]["y"]
        out[b] = (acc.T + bo.astype(np.float64)).astype(np.float32)
    return out



# revision 24
# speedup vs baseline: 1.2165x; 1.2165x over previous
"""Multi-head attention (B=2, S=2048, D=1024, H=16) on 8 TRN2 NeuronCores.

Sharding: data-parallel over batch (2 groups of 4 cores) x head-parallel
(4 heads per core). W_q/W_k/W_v are column-sharded by head, W_o is
row-sharded; the 4 partial W_o outputs per batch are summed on the host
(the unshard step), which also undoes the device-side transposed layout.

Per-core kernel design (bf16 compute, fp32 PSUM accumulation):
  - All tensor operands (x, W_q/k/v/o, qT/kT/v, probs, attn, y) are bf16;
    PSUM accumulates in fp32, softmax denominators/normalisation in fp32.
    End-to-end relative error ~6e-3 (gate is 2e-2).
  - Host pre-transposes X and the weight slices so every matmul
    contraction sits on the partition dim; 1/sqrt(d_k) folded into W_q.
  - q/k projections produce qT/kT in [128 = 2 heads x 64 d, S] layout;
    v in natural [S, d] layout with a ones column per head so the P@V
    matmul accumulates the softmax denominator for free.
  - scores are computed transposed ([k, sq] blocks); softmax skips the
    max-subtraction (scores are O(5), exp accumulated in fp32).
  - causal structure: fully-masked [128 k x 512 sq] blocks are skipped;
    diagonal blocks are truncated to their live column range (width
    512/384/256/128) for scores, exp and P@V; the remaining triangular
    boundary is zeroed with a single shared [128,128] 0/1 mask.
  - denominator reciprocal is broadcast across partitions on the (idle)
    GpSimd engine; the per-g PSUM accumulator is copied to SBUF as soon
    as P@V finishes so the 2-bank PSUM slot frees early.
  - software-pipelined emission: each column's normalisation finish and
    output projection are deferred one column and interleaved as filler
    PE work between attention blocks (which are ACT-exp paced), so the
    PE's in-order queue never head-of-line blocks on the softmax chain.
  - all input DMAs are issued up front so the SP DMA queue never blocks
    input prefetch behind compute-dependent output stores.
"""

import os

import numpy as np

_B, _S, _D, _H, _DK = 2, 2048, 1024, 16, 64
_HPC = 4          # heads per core
_NCORES = 8
_CPG = 4          # cores per (batch) group
_DPC = _HPC * _DK # 256 projection dims per core
_NEG = -1e9

_program_cache = {}
LAST_RESULTS = None  # BassKernelResults of the most recent run (for profiling)
EMIT_LOG = []  # (instruction-id watermark, label) pairs for trace analysis


def _analyze_mask(mask):
    """Classify each [128 k, 512 sq] block of mask^T. Returns (plan, dense).

    plan[i] = tuple of (j, mode, param, off) for sq-tile i; mode 0 = no
    mask, 1 = causal-triangle boundary (masked cells live in columns
    [off, off+128) and satisfy s_rel < p), 2 = dense additive mask
    (param = index into dense blocks). ``off`` is the first live sq
    column of the block (relative to the 512-wide tile); scores/exp/PV
    are restricted to [off, 512). Fully-masked blocks are omitted.
    """
    maskT = np.ascontiguousarray(mask.T)
    plan = []
    dense = []
    p_idx = np.arange(128)[:, None]
    s_idx = np.arange(512)[None, :]
    for i in range(_S // 512):
        row = []
        for j in range(_S // 128):
            blk = maskT[j * 128:(j + 1) * 128, i * 512:(i + 1) * 512]
            nz = blk != 0.0
            if nz.all():
                continue  # fully masked: block contributes nothing
            if not nz.any():
                row.append((j, 0, 0, 0))
                continue
            base = i * 512 - j * 128
            causal = (s_idx + i * 512) < (p_idx + j * 128)
            if np.array_equal(nz, causal) and np.all(blk[nz] == 1.0):
                off = max(0, -base)
                row.append((j, 1, base, off))
            else:
                row.append((j, 2, len(dense), 0))
                dense.append(blk * np.float32(_NEG))
        if row:
            # first block must start at column 0 so the PSUM accumulator
            # is fully initialised by the start=True matmul
            j0, m0, p0, _ = row[0]
            row[0] = (j0, m0, p0, 0)
        plan.append(tuple(row))
    if dense:
        dense_np = np.stack(dense).astype(np.float32)
    else:
        dense_np = np.zeros((1, 128, 512), np.float32)
    return tuple(plan), dense_np


def _build_program(plan, nblk):
    import concourse.bass as bass  # noqa: F401  (registers engine classes)
    import concourse.tile as tile
    from concourse import bacc, mybir

    F32 = mybir.dt.float32
    BF16 = mybir.dt.bfloat16
    AF = mybir.ActivationFunctionType
    ALU = mybir.AluOpType
    ts = bass.ts

    nc = bacc.Bacc(None, target_bir_lowering=False, debug=False)

    xq = nc.dram_tensor("xq", [_D, _S], BF16, kind="ExternalInput").ap()
    xk = nc.dram_tensor("xk", [_D, _S], BF16, kind="ExternalInput").ap()
    xv = nc.dram_tensor("xv", [_D, _S], BF16, kind="ExternalInput").ap()
    wq = nc.dram_tensor("wq", [_D, _DPC], BF16, kind="ExternalInput").ap()
    wk = nc.dram_tensor("wk", [_D, _DPC], BF16, kind="ExternalInput").ap()
    wv = nc.dram_tensor("wv", [_D, _DPC], BF16, kind="ExternalInput").ap()
    wo = nc.dram_tensor("wo", [_DPC, _D], BF16, kind="ExternalInput").ap()
    bq = nc.dram_tensor("bq", [_DPC], F32, kind="ExternalInput").ap()
    bk = nc.dram_tensor("bk", [_DPC], F32, kind="ExternalInput").ap()
    bvb = nc.dram_tensor("bvb", [128, _DPC], F32, kind="ExternalInput").ap()
    mblk = nc.dram_tensor("mblk", [nblk, 128, 512], F32, kind="ExternalInput").ap()
    y = nc.dram_tensor("y", [_D, _S], BF16, kind="ExternalOutput").ap()

    has_dense = any(m == 2 for row in plan for (_, m, _, _) in row)

    EMIT_LOG.clear()

    def mark(lbl):
        EMIT_LOG.append((nc.next_id(), lbl))

    with tile.TileContext(nc) as tc:
        from contextlib import ExitStack
        with ExitStack() as ctx:
            wpool = ctx.enter_context(tc.tile_pool(name="w", bufs=1))
            cpool = ctx.enter_context(tc.tile_pool(name="const", bufs=1))
            xpool = ctx.enter_context(tc.tile_pool(name="xcol", bufs=12))
            biga = ctx.enter_context(tc.tile_pool(name="biga", bufs=1))
            probp = ctx.enter_context(tc.tile_pool(name="probs", bufs=6))
            asbp = ctx.enter_context(tc.tile_pool(name="asb", bufs=4))
            bcp = ctx.enter_context(tc.tile_pool(name="bc", bufs=2))
            yp = ctx.enter_context(tc.tile_pool(name="y", bufs=4))
            mpool = (
                ctx.enter_context(tc.tile_pool(name="mstream", bufs=2))
                if has_dense else None
            )
            mmps = ctx.enter_context(tc.tile_pool(name="mmps", bufs=2, space="PSUM"))
            spsp = ctx.enter_context(tc.tile_pool(name="sps", bufs=2, space="PSUM"))
            accp = ctx.enter_context(tc.tile_pool(name="acc", bufs=1, space="PSUM"))

            xq_r = xq.rearrange("(m p) s -> p m s", p=128)
            xk_r = xk.rearrange("(m p) s -> p m s", p=128)
            xv_r = xv.rearrange("(m p) s -> p m s", p=128)
            wq_r = wq.rearrange("(m p) d -> p m d", p=128)
            wk_r = wk.rearrange("(m p) d -> p m d", p=128)
            wv_r = wv.rearrange("(m p) d -> p m d", p=128)

            # --- all input DMAs issued up front; column 0's q tensors,
            # then the (tiny) biases, then k/v, then the rest.
            bq_sb = cpool.tile([128, 2], F32, tag="bq")
            bk_sb = cpool.tile([128, 2], F32, tag="bk")
            wq_sb = wpool.tile([128, 8, _DPC], BF16, tag="wq")
            wk_sb = wpool.tile([128, 8, _DPC], BF16, tag="wk")
            wv_sb = wpool.tile([128, 8, _DPC], BF16, tag="wv")
            xq_t = [xpool.tile([128, 8, 512], BF16, tag="xcol",
                               name=f"xq_t{c}") for c in range(4)]
            xk_t = [xpool.tile([128, 8, 512], BF16, tag="xcol",
                               name=f"xk_t{c}") for c in range(4)]
            xv_t = [xpool.tile([128, 8, 512], BF16, tag="xcol",
                               name=f"xv_t{c}") for c in range(4)]
            bvb_sb = cpool.tile([128, _DPC], F32, tag="bvb")
            for w_sb, w_r, x_t, x_r in (
                (wq_sb, wq_r, xq_t[0], xq_r),
                (wk_sb, wk_r, xk_t[0], xk_r),
                (wv_sb, wv_r, xv_t[0], xv_r),
            ):
                nc.sync.dma_start(out=w_sb, in_=w_r)
                for lo, hi in ((0, 4), (4, 8)):
                    nc.sync.dma_start(out=x_t[:, lo:hi, :],
                                      in_=x_r[:, lo:hi, ts(0, 512)])
                if w_sb is wq_sb:
                    nc.sync.dma_start(
                        out=bq_sb, in_=bq.rearrange("(h p) -> p h", p=128))
                    nc.sync.dma_start(
                        out=bk_sb, in_=bk.rearrange("(h p) -> p h", p=128))
                if w_sb is wv_sb:
                    nc.sync.dma_start(out=bvb_sb, in_=bvb)

            wo_sb = wpool.tile([128, 2, _D], BF16, tag="wo")

            # remaining x columns (prefetch, in processing order); wo is
            # only needed at the first out-projection (~45us)
            for c in (1, 3, 2):
                nc.sync.dma_start(out=xq_t[c], in_=xq_r[:, :, ts(c, 512)])
                nc.sync.dma_start(out=xk_t[c], in_=xk_r[:, :, ts(c, 512)])
                nc.sync.dma_start(out=xv_t[c], in_=xv_r[:, :, ts(c, 512)])
                if c == 1:
                    nc.sync.dma_start(
                        out=wo_sb, in_=wo.rearrange("(c p) o -> p c o", p=128))

            resident_mask = has_dense and nblk <= 2
            if resident_mask:
                mask_sb = cpool.tile([128, nblk, 512], F32, tag="mask")
                nc.sync.dma_start(
                    out=mask_sb, in_=mblk.rearrange("n p s -> p n s")
                )

            # shared triangular boundary mask: keep cell (p, s_rel) iff
            # s_rel >= p (after live-range shift every causal diagonal
            # block reduces to this)
            use_m01 = any(m == 1 for row in plan for (_, m, _, _) in row)
            if use_m01:
                m01 = cpool.tile([128, 128], BF16, tag="m01")
                nc.vector.memset(m01, 1.0)
                nc.gpsimd.affine_select(
                    out=m01, in_=m01,
                    compare_op=ALU.is_ge, fill=0.0, base=0,
                    channel_multiplier=-1, pattern=[[1, 128]],
                )

            # --- big SBUF state ---
            qT = biga.tile([128, 2, _S], BF16, tag="qT")
            kT = biga.tile([128, 2, _S], BF16, tag="kT")
            vsb = biga.tile([128, 16, _HPC * 65], BF16, tag="v")
            attn = biga.tile([128, 2, _S], BF16, tag="attn")

            # ones columns of v (softmax denominator trick): one strided
            # memset over all 16 x 4 ones columns
            vsb_ones = vsb.rearrange("p s (h x) -> p s h x", x=65)[:, :, :, 64:65]
            nc.vector.memset(vsb_ones, 1.0)

            # v-projection emitted lazily per 512-wide k-column group, the
            # first time any PV needs a chunk from it
            v_pending = set(range(4))

            def ensure_vgroup(col):
                if col not in v_pending:
                    return
                v_pending.discard(col)
                mark(f"vproj{col}")
                for c in range(4):
                    vps = mmps.tile([128, 512], F32, tag="mm", name="vps")
                    for m in range(8):
                        nc.tensor.matmul(
                            vps[:, 0:_DPC], lhsT=xv_t[col][:, m, ts(c, 128)],
                            rhs=wv_sb[:, m, :], start=(m == 0), stop=(m == 7),
                        )
                    sc = col * 4 + c
                    nc.vector.tensor_add(
                        vsb[:, sc, 0:260].rearrange(
                            "p (h x) -> p h x", x=65)[:, :, 0:64],
                        vps[:, 0:_DPC].rearrange("p (h x) -> p h x", x=64),
                        bvb_sb.rearrange("p (h x) -> p h x", x=64),
                    )

            def proj_unit(which, st, dh):
                # one q- or k-projection unit: 8 accumulating matmuls
                # (contraction over D) + DVE bias add into qT/kT
                mark(f"proj_{which}{st}d{dh}")
                x_t, w_sb, b_sb, dst = (
                    (xq_t[st], wq_sb, bq_sb, qT) if which == "q"
                    else (xk_t[st], wk_sb, bk_sb, kT)
                )
                ps = mmps.tile([128, 512], F32, tag="mm", name=f"{which}ps")
                for m in range(8):
                    nc.tensor.matmul(
                        ps, lhsT=w_sb[:, m, ts(dh, 128)], rhs=x_t[:, m, :],
                        start=(m == 0), stop=(m == 7),
                    )
                nc.vector.tensor_scalar(
                    dst[:, dh, ts(st, 512)], ps, b_sb[:, dh:dh + 1], None,
                    ALU.add,
                )

            # deferred work units (FIFO), pumped between attention blocks
            # so the PE always has ready work while exp latency drains.
            # ``reserve`` units are held for the very end of the kernel to
            # fill the PE while the last column's softmax chain drains.
            fillers = []   # entries: (proj_col_or_None, emit_fn)
            reserve = []

            def pump(n=1):
                for _ in range(min(n, len(fillers))):
                    fillers.pop(0)[1]()

            def flush_proj_upto(col):
                # correctness: Tile tracks dependencies in emission order,
                # so any projection for a column whose kT/qT this column's
                # attention reads must be emitted before the blocks
                keep = []
                for ent in fillers:
                    if ent[0] is not None and ent[0] <= col:
                        ent[1]()
                    else:
                        keep.append(ent)
                fillers[:] = keep

            def norm_finish(acc_sb, rec, g, i):
                # broadcast the reciprocal across partitions on GpSimd,
                # then normalise into attn (all-SBUF DVE muls)
                mark(f"normfin_g{g}_c{i}")
                bc_sb = bcp.tile([64, 2, 512], F32, tag="bc", name="bc_sb")
                nc.gpsimd.partition_broadcast(bc_sb, rec, channels=64)
                for hh in range(2):
                    nc.vector.tensor_mul(
                        attn[hh * 64:(hh + 1) * 64, g, ts(i, 512)],
                        acc_sb[0:64, hh, :], bc_sb[:, hh, :],
                    )

            def fast_norm_panel(acc, g, i, lo, hi):
                # per-hh reciprocal/broadcast/multiply for sq columns
                # [lo, hi) straight off the PSUM accumulator
                w = hi - lo
                recl = [None, None]
                for hh in range(2):
                    recl[hh] = bcp.tile([1, 512], F32, tag="recl",
                                        name="recl", bufs=2)
                    with nc.allow_low_precision(
                        reason="softmax reciprocal"
                    ):
                        nc.vector.reciprocal(
                            recl[hh][:, 0:w], acc[64:65, hh, lo:hi])
                bcl = [None, None]
                for hh in range(2):
                    bcl[hh] = bcp.tile([64, 512], F32, tag="bcl",
                                       name="bcl", bufs=2)
                    nc.gpsimd.partition_broadcast(
                        bcl[hh][:, 0:w], recl[hh][:, 0:w], channels=64)
                for hh in range(2):
                    nc.vector.tensor_mul(
                        attn[hh * 64:(hh + 1) * 64, g,
                             i * 512 + lo:i * 512 + hi],
                        acc[0:64, hh, lo:hi], bcl[hh][:, 0:w],
                    )

            def outproj_unit_last(st, oc, pool, act_copy):
                # final-column out-projection, split by sq panel into two
                # independent half-bank PSUM tiles so each half's matmuls,
                # PSUM->SBUF copy and store start as soon as its slice of
                # attn is normalised
                mark(f"outproj{st}_oc{oc}")
                tag = "mm" if pool is mmps else "sps"
                y_sb = yp.tile([128, 512], BF16, tag="y", name="y_sb")
                for lo, hi in ((0, 256), (256, 512)):
                    yps = pool.tile([128, 256], F32, tag=tag, name="yps")
                    for cc in range(2):
                        nc.tensor.matmul(
                            yps, lhsT=wo_sb[:, cc, ts(oc, 128)],
                            rhs=attn[:, cc, st * 512 + lo:st * 512 + hi],
                            start=(cc == 0), stop=(cc == 1),
                        )
                    if act_copy:
                        nc.scalar.copy(y_sb[:, lo:hi], yps)
                    else:
                        nc.vector.tensor_copy(y_sb[:, lo:hi], yps)
                nc.sync.dma_start(
                    out=y[oc * 128:(oc + 1) * 128, ts(st, 512)], in_=y_sb
                )

            def outproj_unit(st, oc, pool=None, act_copy=False,
                             pool_dma=False):
                # output projection for one 128-row slice of y (partial)
                mark(f"outproj{st}_oc{oc}")
                pool = pool or mmps
                tag = "mm" if pool is mmps else "sps"
                yps = pool.tile([128, 512], F32, tag=tag, name="yps")
                for cc in range(2):
                    nc.tensor.matmul(
                        yps, lhsT=wo_sb[:, cc, ts(oc, 128)],
                        rhs=attn[:, cc, ts(st, 512)],
                        start=(cc == 0), stop=(cc == 1),
                    )
                y_sb = yp.tile([128, 512], BF16, tag="y", name="y_sb")
                if act_copy:
                    nc.scalar.copy(y_sb, yps)
                else:
                    nc.vector.tensor_copy(y_sb, yps)
                eng = nc.gpsimd if pool_dma else nc.sync
                eng.dma_start(
                    out=y[oc * 128:(oc + 1) * 128, ts(st, 512)], in_=y_sb
                )

            # --- fused pipeline over sq columns. Processing order puts
            # the largest column (3) third so the deferred work of its
            # predecessor fills its exp-paced bubbles, and column 2 last
            # (its own bubbles absorb column 3's deferred norm/outproj).
            cols_order = (0, 1, 3, 2)
            for idx, st in enumerate(cols_order):
                nxt = cols_order[idx + 1] if idx + 1 < len(cols_order) else None
                last = nxt is None
                if idx == 0:
                    proj_unit("q", st, 0)
                    proj_unit("k", st, 0)
                    fillers.append((0, lambda: proj_unit("q", 0, 1)))
                    fillers.append((0, lambda: proj_unit("k", 0, 1)))
                # queue all remaining columns' projections as filler work
                if idx == 0:
                    for qc in cols_order[1:]:
                        for dh in range(2):
                            fillers.append(
                                (qc,
                                 lambda dh=dh, c=qc: proj_unit("q", c, dh)))
                            fillers.append(
                                (qc,
                                 lambda dh=dh, c=qc: proj_unit("k", c, dh)))
                # correctness: every kT column this column's blocks read
                # must be projected in emission order first
                need = max((b[0] // 4 for b in plan[st]), default=0)
                flush_proj_upto(max(st, need))

                i = st
                blocks = plan[i]
                nj = len(blocks)
                for g in range(2):
                    acc = accp.tile([65, 2, 512], F32, tag="acc",
                                    name=f"acc{st}{g}")

                    def emit_pv(pend, start, stop):
                        j_p, off_p, probs_p = pend
                        for hh in range(2):
                            h = 2 * g + hh
                            nc.tensor.matmul(
                                acc[:, hh, off_p:512],
                                lhsT=vsb[:, j_p, h * 65:(h + 1) * 65],
                                rhs=probs_p[:, hh, off_p:512],
                                start=start, stop=stop,
                            )

                    tail_g = last and g == 1
                    if tail_g:
                        # flush deferred work now so its DVE traffic lands
                        # ahead of the softmax-chain ops in the DVE queue
                        while fillers:
                            pump(1)
                        for r in reserve:
                            r()
                    pend = None  # scores/exp run one block ahead of PV
                    for bi, (j, mode, param, off) in enumerate(blocks):
                        mark(f"blk_c{st}g{g}j{j}")
                        ensure_vgroup(j // 4)
                        sps = spsp.tile([128, 2, 512], F32, tag="sps", name="sps")
                        for hh in range(2):
                            nc.tensor.matmul(
                                sps[:, hh, off:512],
                                lhsT=kT[hh * 64:(hh + 1) * 64, g, ts(j, 128)],
                                rhs=qT[hh * 64:(hh + 1) * 64, g,
                                       i * 512 + off:(i + 1) * 512],
                                start=True, stop=True,
                            )
                        if mode == 2:
                            if resident_mask:
                                mt = mask_sb[:, param, :]
                            else:
                                mt = mpool.tile([128, 512], F32, tag="mtile",
                                                name="mt")
                                nc.sync.dma_start(out=mt, in_=mblk[param])
                            for hh in range(2):
                                nc.vector.tensor_add(
                                    sps[:, hh, :], sps[:, hh, :], mt
                                )
                        probs = probp.tile([128, 2, 512], BF16, tag="probs",
                                           name="probs")
                        nc.scalar.activation(
                            probs[:, :, off:512], sps[:, :, off:512], AF.Exp
                        )
                        if mode == 1:
                            # masked cells sit in columns [off, off+128):
                            # s_rel < p relative to the live window
                            for hh in range(2):
                                nc.vector.tensor_mul(
                                    probs[:, hh, off:off + 128],
                                    probs[:, hh, off:off + 128],
                                    m01,
                                )
                        if pend is not None:
                            emit_pv(pend, start=(bi == 1), stop=False)
                            pump(1)
                        pend = (j, off, probs)
                    emit_pv(pend, start=(nj == 1), stop=True)
                    if tail_g:
                        fast_norm_panel(acc, g, i, 0, 512)
                    pump(1)

                    if not tail_g:
                        # copy the accumulator out of PSUM promptly (frees
                        # the 2-bank slot for the other head-pair group) and
                        # take the reciprocal; the rest of the normalisation
                        # is deferred as filler work
                        mark(f"acccopy_c{st}g{g}")
                        acc_sb = asbp.tile([65, 2, 512], F32, tag="asb",
                                           name="acc_sb")
                        nc.vector.tensor_copy(acc_sb, acc)
                        rec = bcp.tile([1, 2, 512], F32, tag="rec",
                                       name="rec", bufs=3)
                        with nc.allow_low_precision(
                            reason="softmax reciprocal"
                        ):
                            nc.vector.reciprocal(rec, acc_sb[64:65, :, :])
                        if not last:
                            fillers.append(
                                (None, lambda a=acc_sb, r=rec, g=g, i=i:
                                 norm_finish(a, r, g, i)))
                            if g == 1:
                                for oc in range(8):
                                    item = (None,
                                            lambda st=st, oc=oc:
                                            outproj_unit(st, oc))
                                    if (idx == len(cols_order) - 2
                                            and oc >= 4):
                                        reserve.append(item[1])
                                    else:
                                        fillers.append(item)
                        else:
                            norm_finish(acc_sb, rec, g, i)

                if last:
                    for oc in range(8):
                        outproj_unit(st, oc,
                                     pool=(spsp if oc % 2 else mmps),
                                     act_copy=bool(oc % 2))

            while fillers:
                pump(1)

    nc.compile()
    return nc


def kernel(**inputs):
    global LAST_RESULTS
    from concourse.bass_utils import run_bass_kernel_spmd

    Q = np.asarray(inputs["Q"], dtype=np.float32)
    K = np.asarray(inputs["K"], dtype=np.float32)
    V = np.asarray(inputs["V"], dtype=np.float32)
    mask = np.asarray(inputs["mask"], dtype=np.float32)
    Wq = np.asarray(inputs["Wq"], dtype=np.float32)
    bq = np.asarray(inputs["bq"], dtype=np.float32)
    Wk = np.asarray(inputs["Wk"], dtype=np.float32)
    bk = np.asarray(inputs["bk"], dtype=np.float32)
    Wv = np.asarray(inputs["Wv"], dtype=np.float32)
    bv = np.asarray(inputs["bv"], dtype=np.float32)
    Wo = np.asarray(inputs["Wo"], dtype=np.float32)
    bo = np.asarray(inputs["bo"], dtype=np.float32)

    plan, dense = _analyze_mask(mask)
    key = (plan, dense.shape[0])
    if key not in _program_cache:
        _program_cache[key] = _build_program(plan, dense.shape[0])
    nc = _program_cache[key]

    import ml_dtypes
    bf16 = ml_dtypes.bfloat16
    sc = np.float32(1.0 / np.sqrt(_DK))
    xqT = [np.ascontiguousarray(Q[b].T).astype(bf16) for b in range(_B)]
    xkT = [np.ascontiguousarray(K[b].T).astype(bf16) for b in range(_B)]
    xvT = [np.ascontiguousarray(V[b].T).astype(bf16) for b in range(_B)]

    in_maps = []
    for core in range(_NCORES):
        b = core // _CPG
        rows = slice((core % _CPG) * _DPC, (core % _CPG) * _DPC + _DPC)
        in_maps.append({
            "xq": xqT[b], "xk": xkT[b], "xv": xvT[b],
            "wq": np.ascontiguousarray((Wq[rows] * sc).T).astype(bf16),
            "wk": np.ascontiguousarray(Wk[rows].T).astype(bf16),
            "wv": np.ascontiguousarray(Wv[rows].T).astype(bf16),
            "wo": np.ascontiguousarray(Wo[:, rows].T).astype(bf16),
            "bq": np.ascontiguousarray(bq[rows] * sc),
            "bk": np.ascontiguousarray(bk[rows]),
            "bvb": np.broadcast_to(bv[rows], (128, _DPC)).copy(),
            "mblk": dense,
        })

    trace = bool(int(os.environ.get("KERNEL_TRACE", "0")))
    LAST_RESULTS = run_bass_kernel_spmd(
        nc, in_maps, list(range(_NCORES)), trace=trace
    )

    out = np.empty((_B, _S, _D), np.float32)
    for b in range(_B):
        acc = np.zeros((_D, _S), np.float32)
        for c in range(_CPG):
            acc += LAST_RESULTS.results[b * _CPG + c]["y"].astype(np.float32)
        out[b] = (acc.T + bo).astype(np.float32)
    return out


# revision 27
# speedup vs baseline: 1.2203x; 1.0031x over previous
"""Multi-head attention (B=2, S=2048, D=1024, H=16) on 8 TRN2 NeuronCores.

Sharding: data-parallel over batch (2 groups of 4 cores) x head-parallel
(4 heads per core). W_q/W_k/W_v are column-sharded by head, W_o is
row-sharded; the 4 partial W_o outputs per batch are summed on the host
(the unshard step), which also undoes the device-side transposed layout.

Per-core kernel design (bf16 compute, fp32 PSUM accumulation):
  - All tensor operands (x, W_q/k/v/o, qT/kT/v, probs, attn, y) are bf16;
    PSUM accumulates in fp32, softmax denominators/normalisation in fp32.
    End-to-end relative error ~6e-3 (gate is 2e-2).
  - Host pre-transposes X and the weight slices so every matmul
    contraction sits on the partition dim; 1/sqrt(d_k) folded into W_q.
  - q/k projections produce qT/kT in [128 = 2 heads x 64 d, S] layout;
    v in natural [S, d] layout with a ones column per head so the P@V
    matmul accumulates the softmax denominator for free.
  - scores are computed transposed ([k, sq] blocks); softmax skips the
    max-subtraction (scores are O(5), exp accumulated in fp32).
  - causal structure: fully-masked [128 k x 512 sq] blocks are skipped;
    diagonal blocks are truncated to their live column range (width
    512/384/256/128) for scores, exp and P@V; the remaining triangular
    boundary is zeroed with a single shared [128,128] 0/1 mask.
  - denominator reciprocal is broadcast across partitions on the (idle)
    GpSimd engine; the per-g PSUM accumulator is copied to SBUF as soon
    as P@V finishes so the 2-bank PSUM slot frees early.
  - software-pipelined emission: each column's normalisation finish and
    output projection are deferred one column and interleaved as filler
    PE work between attention blocks (which are ACT-exp paced), so the
    PE's in-order queue never head-of-line blocks on the softmax chain.
  - all input DMAs are issued up front so the SP DMA queue never blocks
    input prefetch behind compute-dependent output stores.
"""

import os

import numpy as np

_B, _S, _D, _H, _DK = 2, 2048, 1024, 16, 64
_HPC = 4          # heads per core
_NCORES = 8
_CPG = 4          # cores per (batch) group
_DPC = _HPC * _DK # 256 projection dims per core
_NEG = -1e9

_program_cache = {}
LAST_RESULTS = None  # BassKernelResults of the most recent run (for profiling)
EMIT_LOG = []  # (instruction-id watermark, label) pairs for trace analysis


def _analyze_mask(mask):
    """Classify each [128 k, 512 sq] block of mask^T. Returns (plan, dense).

    plan[i] = tuple of (j, mode, param, off) for sq-tile i; mode 0 = no
    mask, 1 = causal-triangle boundary (masked cells live in columns
    [off, off+128) and satisfy s_rel < p), 2 = dense additive mask
    (param = index into dense blocks). ``off`` is the first live sq
    column of the block (relative to the 512-wide tile); scores/exp/PV
    are restricted to [off, 512). Fully-masked blocks are omitted.
    """
    maskT = np.ascontiguousarray(mask.T)
    plan = []
    dense = []
    p_idx = np.arange(128)[:, None]
    s_idx = np.arange(512)[None, :]
    for i in range(_S // 512):
        row = []
        for j in range(_S // 128):
            blk = maskT[j * 128:(j + 1) * 128, i * 512:(i + 1) * 512]
            nz = blk != 0.0
            if nz.all():
                continue  # fully masked: block contributes nothing
            if not nz.any():
                row.append((j, 0, 0, 0))
                continue
            base = i * 512 - j * 128
            causal = (s_idx + i * 512) < (p_idx + j * 128)
            if np.array_equal(nz, causal) and np.all(blk[nz] == 1.0):
                off = max(0, -base)
                row.append((j, 1, base, off))
            else:
                row.append((j, 2, len(dense), 0))
                dense.append(blk * np.float32(_NEG))
        if row:
            # first block must start at column 0 so the PSUM accumulator
            # is fully initialised by the start=True matmul
            j0, m0, p0, _ = row[0]
            row[0] = (j0, m0, p0, 0)
        plan.append(tuple(row))
    if dense:
        dense_np = np.stack(dense).astype(np.float32)
    else:
        dense_np = np.zeros((1, 128, 512), np.float32)
    return tuple(plan), dense_np


def _build_program(plan, nblk):
    import concourse.bass as bass  # noqa: F401  (registers engine classes)
    import concourse.tile as tile
    from concourse import bacc, mybir

    F32 = mybir.dt.float32
    BF16 = mybir.dt.bfloat16
    AF = mybir.ActivationFunctionType
    ALU = mybir.AluOpType
    ts = bass.ts

    nc = bacc.Bacc(None, target_bir_lowering=False, debug=False)

    xq = nc.dram_tensor("xq", [_D, _S], BF16, kind="ExternalInput").ap()
    xk = nc.dram_tensor("xk", [_D, _S], BF16, kind="ExternalInput").ap()
    xv = nc.dram_tensor("xv", [_D, _S], BF16, kind="ExternalInput").ap()
    wq = nc.dram_tensor("wq", [_D, _DPC], BF16, kind="ExternalInput").ap()
    wk = nc.dram_tensor("wk", [_D, _DPC], BF16, kind="ExternalInput").ap()
    wv = nc.dram_tensor("wv", [_D, _DPC], BF16, kind="ExternalInput").ap()
    wo = nc.dram_tensor("wo", [_DPC, _D], BF16, kind="ExternalInput").ap()
    bq = nc.dram_tensor("bq", [_DPC], F32, kind="ExternalInput").ap()
    bk = nc.dram_tensor("bk", [_DPC], F32, kind="ExternalInput").ap()
    bvb = nc.dram_tensor("bvb", [128, _DPC], F32, kind="ExternalInput").ap()
    mblk = nc.dram_tensor("mblk", [nblk, 128, 512], F32, kind="ExternalInput").ap()
    y = nc.dram_tensor("y", [_D, _S], BF16, kind="ExternalOutput").ap()

    has_dense = any(m == 2 for row in plan for (_, m, _, _) in row)

    EMIT_LOG.clear()

    def mark(lbl):
        EMIT_LOG.append((nc.next_id(), lbl))

    with tile.TileContext(nc) as tc:
        from contextlib import ExitStack
        with ExitStack() as ctx:
            wpool = ctx.enter_context(tc.tile_pool(name="w", bufs=1))
            cpool = ctx.enter_context(tc.tile_pool(name="const", bufs=1))
            xpool = ctx.enter_context(tc.tile_pool(name="xcol", bufs=12))
            biga = ctx.enter_context(tc.tile_pool(name="biga", bufs=1))
            probp = ctx.enter_context(tc.tile_pool(name="probs", bufs=6))
            asbp = ctx.enter_context(tc.tile_pool(name="asb", bufs=4))
            bcp = ctx.enter_context(tc.tile_pool(name="bc", bufs=2))
            yp = ctx.enter_context(tc.tile_pool(name="y", bufs=4))
            mpool = (
                ctx.enter_context(tc.tile_pool(name="mstream", bufs=2))
                if has_dense else None
            )
            mmps = ctx.enter_context(tc.tile_pool(name="mmps", bufs=2, space="PSUM"))
            spsp = ctx.enter_context(tc.tile_pool(name="sps", bufs=2, space="PSUM"))
            accp = ctx.enter_context(tc.tile_pool(name="acc", bufs=2, space="PSUM"))

            xq_r = xq.rearrange("(m p) s -> p m s", p=128)
            xk_r = xk.rearrange("(m p) s -> p m s", p=128)
            xv_r = xv.rearrange("(m p) s -> p m s", p=128)
            wq_r = wq.rearrange("(m p) d -> p m d", p=128)
            wk_r = wk.rearrange("(m p) d -> p m d", p=128)
            wv_r = wv.rearrange("(m p) d -> p m d", p=128)

            # --- all input DMAs issued up front; column 0's q tensors,
            # then the (tiny) biases, then k/v, then the rest.
            bq_sb = cpool.tile([128, 2], F32, tag="bq")
            bk_sb = cpool.tile([128, 2], F32, tag="bk")
            wq_sb = wpool.tile([128, 8, _DPC], BF16, tag="wq")
            wk_sb = wpool.tile([128, 8, _DPC], BF16, tag="wk")
            wv_sb = wpool.tile([128, 8, _DPC], BF16, tag="wv")
            xq_t = [xpool.tile([128, 8, 512], BF16, tag="xcol",
                               name=f"xq_t{c}") for c in range(4)]
            xk_t = [xpool.tile([128, 8, 512], BF16, tag="xcol",
                               name=f"xk_t{c}") for c in range(4)]
            xv_t = [xpool.tile([128, 8, 512], BF16, tag="xcol",
                               name=f"xv_t{c}") for c in range(4)]
            bvb_sb = cpool.tile([128, _DPC], F32, tag="bvb")
            for w_sb, w_r, x_t, x_r in (
                (wq_sb, wq_r, xq_t[0], xq_r),
                (wk_sb, wk_r, xk_t[0], xk_r),
                (wv_sb, wv_r, xv_t[0], xv_r),
            ):
                nc.sync.dma_start(out=w_sb, in_=w_r)
                for lo, hi in ((0, 4), (4, 8)):
                    nc.sync.dma_start(out=x_t[:, lo:hi, :],
                                      in_=x_r[:, lo:hi, ts(0, 512)])
                if w_sb is wq_sb:
                    nc.sync.dma_start(
                        out=bq_sb, in_=bq.rearrange("(h p) -> p h", p=128))
                    nc.sync.dma_start(
                        out=bk_sb, in_=bk.rearrange("(h p) -> p h", p=128))
                if w_sb is wv_sb:
                    nc.sync.dma_start(out=bvb_sb, in_=bvb)

            wo_sb = wpool.tile([128, 2, _D], BF16, tag="wo")

            # remaining x columns (prefetch, in processing order); wo is
            # only needed at the first out-projection (~45us)
            for c in (1, 3, 2):
                nc.sync.dma_start(out=xq_t[c], in_=xq_r[:, :, ts(c, 512)])
                nc.sync.dma_start(out=xk_t[c], in_=xk_r[:, :, ts(c, 512)])
                nc.sync.dma_start(out=xv_t[c], in_=xv_r[:, :, ts(c, 512)])
                if c == 1:
                    nc.sync.dma_start(
                        out=wo_sb, in_=wo.rearrange("(c p) o -> p c o", p=128))

            resident_mask = has_dense and nblk <= 2
            if resident_mask:
                mask_sb = cpool.tile([128, nblk, 512], F32, tag="mask")
                nc.sync.dma_start(
                    out=mask_sb, in_=mblk.rearrange("n p s -> p n s")
                )

            # shared triangular boundary mask: keep cell (p, s_rel) iff
            # s_rel >= p (after live-range shift every causal diagonal
            # block reduces to this)
            use_m01 = any(m == 1 for row in plan for (_, m, _, _) in row)
            if use_m01:
                m01 = cpool.tile([128, 128], BF16, tag="m01")
                nc.vector.memset(m01, 1.0)
                nc.gpsimd.affine_select(
                    out=m01, in_=m01,
                    compare_op=ALU.is_ge, fill=0.0, base=0,
                    channel_multiplier=-1, pattern=[[1, 128]],
                )

            # --- big SBUF state ---
            qT = biga.tile([128, 2, _S], BF16, tag="qT")
            kT = biga.tile([128, 2, _S], BF16, tag="kT")
            vsb = biga.tile([128, 16, _HPC * 65], BF16, tag="v")
            attn = biga.tile([128, 2, _S], BF16, tag="attn")

            # ones columns of v (softmax denominator trick): one strided
            # memset over all 16 x 4 ones columns
            vsb_ones = vsb.rearrange("p s (h x) -> p s h x", x=65)[:, :, :, 64:65]
            nc.vector.memset(vsb_ones, 1.0)

            # v-projection emitted lazily per 512-wide k-column group, the
            # first time any PV needs a chunk from it
            v_pending = set(range(4))

            def ensure_vgroup(col):
                if col not in v_pending:
                    return
                v_pending.discard(col)
                mark(f"vproj{col}")
                for c in range(4):
                    vps = mmps.tile([128, 512], F32, tag="mm", name="vps")
                    for m in range(8):
                        nc.tensor.matmul(
                            vps[:, 0:_DPC], lhsT=xv_t[col][:, m, ts(c, 128)],
                            rhs=wv_sb[:, m, :], start=(m == 0), stop=(m == 7),
                        )
                    sc = col * 4 + c
                    nc.vector.tensor_add(
                        vsb[:, sc, 0:260].rearrange(
                            "p (h x) -> p h x", x=65)[:, :, 0:64],
                        vps[:, 0:_DPC].rearrange("p (h x) -> p h x", x=64),
                        bvb_sb.rearrange("p (h x) -> p h x", x=64),
                    )

            def proj_unit(which, st, dh):
                # one q- or k-projection unit: 8 accumulating matmuls
                # (contraction over D) + DVE bias add into qT/kT
                mark(f"proj_{which}{st}d{dh}")
                x_t, w_sb, b_sb, dst = (
                    (xq_t[st], wq_sb, bq_sb, qT) if which == "q"
                    else (xk_t[st], wk_sb, bk_sb, kT)
                )
                ps = mmps.tile([128, 512], F32, tag="mm", name=f"{which}ps")
                for m in range(8):
                    nc.tensor.matmul(
                        ps, lhsT=w_sb[:, m, ts(dh, 128)], rhs=x_t[:, m, :],
                        start=(m == 0), stop=(m == 7),
                    )
                nc.vector.tensor_scalar(
                    dst[:, dh, ts(st, 512)], ps, b_sb[:, dh:dh + 1], None,
                    ALU.add,
                )

            # deferred work units (FIFO), pumped between attention blocks
            # so the PE always has ready work while exp latency drains.
            # ``reserve`` units are held for the very end of the kernel to
            # fill the PE while the last column's softmax chain drains.
            fillers = []   # entries: (proj_col_or_None, emit_fn)
            reserve = []

            def pump(n=1):
                for _ in range(min(n, len(fillers))):
                    fillers.pop(0)[1]()

            def flush_proj_upto(col):
                # correctness: Tile tracks dependencies in emission order,
                # so any projection for a column whose kT/qT this column's
                # attention reads must be emitted before the blocks
                keep = []
                for ent in fillers:
                    if ent[0] is not None and ent[0] <= col:
                        ent[1]()
                    else:
                        keep.append(ent)
                fillers[:] = keep

            def norm_finish(acc_sb, rec, g, i):
                # broadcast the reciprocal across partitions on GpSimd,
                # then normalise into attn (all-SBUF DVE muls)
                mark(f"normfin_g{g}_c{i}")
                bc_sb = bcp.tile([64, 2, 512], F32, tag="bc", name="bc_sb")
                nc.gpsimd.partition_broadcast(bc_sb, rec, channels=64)
                for hh in range(2):
                    nc.vector.tensor_mul(
                        attn[hh * 64:(hh + 1) * 64, g, ts(i, 512)],
                        acc_sb[0:64, hh, :], bc_sb[:, hh, :],
                    )

            def fast_norm_panel(acc, g, i, lo, hi):
                # per-hh reciprocal/broadcast/multiply for sq columns
                # [lo, hi) straight off the PSUM accumulator
                w = hi - lo
                recl = [None, None]
                for hh in range(2):
                    recl[hh] = bcp.tile([1, 512], F32, tag="recl",
                                        name="recl", bufs=2)
                    with nc.allow_low_precision(
                        reason="softmax reciprocal"
                    ):
                        nc.vector.reciprocal(
                            recl[hh][:, 0:w], acc[hh][64:65, lo:hi])
                bcl = [None, None]
                for hh in range(2):
                    bcl[hh] = bcp.tile([64, 512], F32, tag="bcl",
                                       name="bcl", bufs=2)
                    nc.gpsimd.partition_broadcast(
                        bcl[hh][:, 0:w], recl[hh][:, 0:w], channels=64)
                for hh in range(2):
                    nc.vector.tensor_mul(
                        attn[hh * 64:(hh + 1) * 64, g,
                             i * 512 + lo:i * 512 + hi],
                        acc[hh][0:64, lo:hi], bcl[hh][:, 0:w],
                    )

            def outproj_unit_last(st, oc, pool, act_copy):
                # final-column out-projection, split by sq panel into two
                # independent half-bank PSUM tiles so each half's matmuls,
                # PSUM->SBUF copy and store start as soon as its slice of
                # attn is normalised
                mark(f"outproj{st}_oc{oc}")
                tag = "mm" if pool is mmps else "sps"
                y_sb = yp.tile([128, 512], BF16, tag="y", name="y_sb")
                for lo, hi in ((0, 256), (256, 512)):
                    yps = pool.tile([128, 256], F32, tag=tag, name="yps")
                    for cc in range(2):
                        nc.tensor.matmul(
                            yps, lhsT=wo_sb[:, cc, ts(oc, 128)],
                            rhs=attn[:, cc, st * 512 + lo:st * 512 + hi],
                            start=(cc == 0), stop=(cc == 1),
                        )
                    if act_copy:
                        nc.scalar.copy(y_sb[:, lo:hi], yps)
                    else:
                        nc.vector.tensor_copy(y_sb[:, lo:hi], yps)
                nc.sync.dma_start(
                    out=y[oc * 128:(oc + 1) * 128, ts(st, 512)], in_=y_sb
                )

            def outproj_unit(st, oc, pool=None, act_copy=False,
                             pool_dma=False):
                # output projection for one 128-row slice of y (partial)
                mark(f"outproj{st}_oc{oc}")
                pool = pool or mmps
                tag = "mm" if pool is mmps else "sps"
                yps = pool.tile([128, 512], F32, tag=tag, name="yps")
                for cc in range(2):
                    nc.tensor.matmul(
                        yps, lhsT=wo_sb[:, cc, ts(oc, 128)],
                        rhs=attn[:, cc, ts(st, 512)],
                        start=(cc == 0), stop=(cc == 1),
                    )
                y_sb = yp.tile([128, 512], BF16, tag="y", name="y_sb")
                if act_copy:
                    nc.scalar.copy(y_sb, yps)
                else:
                    nc.vector.tensor_copy(y_sb, yps)
                eng = nc.gpsimd if pool_dma else nc.sync
                eng.dma_start(
                    out=y[oc * 128:(oc + 1) * 128, ts(st, 512)], in_=y_sb
                )

            # --- fused pipeline over sq columns. Processing order puts
            # the largest column (3) third so the deferred work of its
            # predecessor fills its exp-paced bubbles, and column 2 last
            # (its own bubbles absorb column 3's deferred norm/outproj).
            cols_order = (0, 1, 3, 2)
            for idx, st in enumerate(cols_order):
                nxt = cols_order[idx + 1] if idx + 1 < len(cols_order) else None
                last = nxt is None
                if idx == 0:
                    proj_unit("q", st, 0)
                    proj_unit("k", st, 0)
                    fillers.append((0, lambda: proj_unit("q", 0, 1)))
                    fillers.append((0, lambda: proj_unit("k", 0, 1)))
                # queue all remaining columns' projections as filler work
                if idx == 0:
                    for qc in cols_order[1:]:
                        for dh in range(2):
                            fillers.append(
                                (qc,
                                 lambda dh=dh, c=qc: proj_unit("q", c, dh)))
                            fillers.append(
                                (qc,
                                 lambda dh=dh, c=qc: proj_unit("k", c, dh)))
                # correctness: every kT column this column's blocks read
                # must be projected in emission order first
                need = max((b[0] // 4 for b in plan[st]), default=0)
                flush_proj_upto(max(st, need))

                i = st
                blocks = plan[i]
                nj = len(blocks)
                for g in range(2):
                    # two independent 1-bank accumulators (one per hh) so
                    # the next group's P@V unblocks per-half as each copy
                    # drains, instead of waiting for the full 2-bank tile
                    acc = [accp.tile([65, 512], F32, tag="acc",
                                     name=f"acc{st}{g}h{hh}")
                           for hh in range(2)]

                    def emit_pv(pend, start, stop):
                        j_p, off_p, probs_p = pend
                        for hh in range(2):
                            h = 2 * g + hh
                            nc.tensor.matmul(
                                acc[hh][:, off_p:512],
                                lhsT=vsb[:, j_p, h * 65:(h + 1) * 65],
                                rhs=probs_p[:, hh, off_p:512],
                                start=start, stop=stop,
                            )

                    tail_g = last and g == 1
                    if tail_g:
                        # flush deferred work now so its DVE traffic lands
                        # ahead of the softmax-chain ops in the DVE queue
                        while fillers:
                            pump(1)
                        for r in reserve:
                            r()
                    pend = None  # scores/exp run one block ahead of PV
                    for bi, (j, mode, param, off) in enumerate(blocks):
                        mark(f"blk_c{st}g{g}j{j}")
                        ensure_vgroup(j // 4)
                        sps = spsp.tile([128, 2, 512], F32, tag="sps", name="sps")
                        for hh in range(2):
                            nc.tensor.matmul(
                                sps[:, hh, off:512],
                                lhsT=kT[hh * 64:(hh + 1) * 64, g, ts(j, 128)],
                                rhs=qT[hh * 64:(hh + 1) * 64, g,
                                       i * 512 + off:(i + 1) * 512],
                                start=True, stop=True,
                            )
                        if mode == 2:
                            if resident_mask:
                                mt = mask_sb[:, param, :]
                            else:
                                mt = mpool.tile([128, 512], F32, tag="mtile",
                                                name="mt")
                                nc.sync.dma_start(out=mt, in_=mblk[param])
                            for hh in range(2):
                                nc.vector.tensor_add(
                                    sps[:, hh, :], sps[:, hh, :], mt
                                )
                        probs = probp.tile([128, 2, 512], BF16, tag="probs",
                                           name="probs")
                        nc.scalar.activation(
                            probs[:, :, off:512], sps[:, :, off:512], AF.Exp
                        )
                        if mode == 1:
                            # masked cells sit in columns [off, off+128):
                            # s_rel < p relative to the live window
                            for hh in range(2):
                                nc.vector.tensor_mul(
                                    probs[:, hh, off:off + 128],
                                    probs[:, hh, off:off + 128],
                                    m01,
                                )
                        if pend is not None:
                            emit_pv(pend, start=(bi == 1), stop=False)
                            pump(1)
                        pend = (j, off, probs)
                    emit_pv(pend, start=(nj == 1), stop=True)
                    if tail_g:
                        fast_norm_panel(acc, g, i, 0, 512)
                    pump(2)

                    if not tail_g:
                        # copy the accumulator out of PSUM promptly (frees
                        # the 2-bank slot for the other head-pair group) and
                        # take the reciprocal; the rest of the normalisation
                        # is deferred as filler work
                        mark(f"acccopy_c{st}g{g}")
                        acc_sb = asbp.tile([65, 2, 512], F32, tag="asb",
                                           name="acc_sb")
                        for hh in range(2):
                            nc.vector.tensor_copy(acc_sb[:, hh, :], acc[hh])
                        rec = bcp.tile([1, 2, 512], F32, tag="rec",
                                       name="rec", bufs=3)
                        with nc.allow_low_precision(
                            reason="softmax reciprocal"
                        ):
                            nc.vector.reciprocal(rec, acc_sb[64:65, :, :])
                        if not last:
                            fillers.append(
                                (None, lambda a=acc_sb, r=rec, g=g, i=i:
                                 norm_finish(a, r, g, i)))
                            if g == 1:
                                for oc in range(8):
                                    item = (None,
                                            lambda st=st, oc=oc:
                                            outproj_unit(st, oc))
                                    if (idx == len(cols_order) - 2
                                            and oc >= 4):
                                        reserve.append(item[1])
                                    else:
                                        fillers.append(item)
                        else:
                            norm_finish(acc_sb, rec, g, i)

                if last:
                    for oc in range(8):
                        outproj_unit(st, oc,
                                     pool=(spsp if oc % 2 else mmps),
                                     act_copy=bool(oc % 2))

            while fillers:
                pump(1)

    nc.compile()
    return nc


def kernel(**inputs):
    global LAST_RESULTS
    from concourse.bass_utils import run_bass_kernel_spmd

    Q = np.asarray(inputs["Q"], dtype=np.float32)
    K = np.asarray(inputs["K"], dtype=np.float32)
    V = np.asarray(inputs["V"], dtype=np.float32)
    mask = np.asarray(inputs["mask"], dtype=np.float32)
    Wq = np.asarray(inputs["Wq"], dtype=np.float32)
    bq = np.asarray(inputs["bq"], dtype=np.float32)
    Wk = np.asarray(inputs["Wk"], dtype=np.float32)
    bk = np.asarray(inputs["bk"], dtype=np.float32)
    Wv = np.asarray(inputs["Wv"], dtype=np.float32)
    bv = np.asarray(inputs["bv"], dtype=np.float32)
    Wo = np.asarray(inputs["Wo"], dtype=np.float32)
    bo = np.asarray(inputs["bo"], dtype=np.float32)

    plan, dense = _analyze_mask(mask)
    key = (plan, dense.shape[0])
    if key not in _program_cache:
        _program_cache[key] = _build_program(plan, dense.shape[0])
    nc = _program_cache[key]

    import ml_dtypes
    bf16 = ml_dtypes.bfloat16
    sc = np.float32(1.0 / np.sqrt(_DK))
    xqT = [np.ascontiguousarray(Q[b].T).astype(bf16) for b in range(_B)]
    xkT = [np.ascontiguousarray(K[b].T).astype(bf16) for b in range(_B)]
    xvT = [np.ascontiguousarray(V[b].T).astype(bf16) for b in range(_B)]

    in_maps = []
    for core in range(_NCORES):
        b = core // _CPG
        rows = slice((core % _CPG) * _DPC, (core % _CPG) * _DPC + _DPC)
        in_maps.append({
            "xq": xqT[b], "xk": xkT[b], "xv": xvT[b],
            "wq": np.ascontiguousarray((Wq[rows] * sc).T).astype(bf16),
            "wk": np.ascontiguousarray(Wk[rows].T).astype(bf16),
            "wv": np.ascontiguousarray(Wv[rows].T).astype(bf16),
            "wo": np.ascontiguousarray(Wo[:, rows].T).astype(bf16),
            "bq": np.ascontiguousarray(bq[rows] * sc),
            "bk": np.ascontiguousarray(bk[rows]),
            "bvb": np.broadcast_to(bv[rows], (128, _DPC)).copy(),
            "mblk": dense,
        })

    trace = bool(int(os.environ.get("KERNEL_TRACE", "0")))
    LAST_RESULTS = run_bass_kernel_spmd(
        nc, in_maps, list(range(_NCORES)), trace=trace
    )

    out = np.empty((_B, _S, _D), np.float32)
    for b in range(_B):
        acc = np.zeros((_D, _S), np.float32)
        for c in range(_CPG):
            acc += LAST_RESULTS.results[b * _CPG + c]["y"].astype(np.float32)
        out[b] = (acc.T + bo).astype(np.float32)
    return out


# revision 28
# speedup vs baseline: 1.2283x; 1.0066x over previous
"""Multi-head attention (B=2, S=2048, D=1024, H=16) on 8 TRN2 NeuronCores.

Sharding: data-parallel over batch (2 groups of 4 cores) x head-parallel
(4 heads per core). W_q/W_k/W_v are column-sharded by head, W_o is
row-sharded; the 4 partial W_o outputs per batch are summed on the host
(the unshard step), which also undoes the device-side transposed layout.

Per-core kernel design (bf16 compute, fp32 PSUM accumulation):
  - All tensor operands (x, W_q/k/v/o, qT/kT/v, probs, attn, y) are bf16;
    PSUM accumulates in fp32, softmax denominators/normalisation in fp32.
    End-to-end relative error ~6e-3 (gate is 2e-2).
  - Host pre-transposes X and the weight slices so every matmul
    contraction sits on the partition dim; 1/sqrt(d_k) folded into W_q.
  - q/k projections produce qT/kT in [128 = 2 heads x 64 d, S] layout;
    v in natural [S, d] layout with a ones column per head so the P@V
    matmul accumulates the softmax denominator for free.
  - scores are computed transposed ([k, sq] blocks); softmax skips the
    max-subtraction (scores are O(5), exp accumulated in fp32).
  - causal structure: fully-masked [128 k x 512 sq] blocks are skipped;
    diagonal blocks are truncated to their live column range (width
    512/384/256/128) for scores, exp and P@V; the remaining triangular
    boundary is zeroed with a single shared [128,128] 0/1 mask.
  - denominator reciprocal is broadcast across partitions on the (idle)
    GpSimd engine; the per-g PSUM accumulator is copied to SBUF as soon
    as P@V finishes so the 2-bank PSUM slot frees early.
  - software-pipelined emission: each column's normalisation finish and
    output projection are deferred one column and interleaved as filler
    PE work between attention blocks (which are ACT-exp paced), so the
    PE's in-order queue never head-of-line blocks on the softmax chain.
  - all input DMAs are issued up front so the SP DMA queue never blocks
    input prefetch behind compute-dependent output stores.
"""

import os

import numpy as np

_B, _S, _D, _H, _DK = 2, 2048, 1024, 16, 64
_HPC = 4          # heads per core
_NCORES = 8
_CPG = 4          # cores per (batch) group
_DPC = _HPC * _DK # 256 projection dims per core
_NEG = -1e9

_program_cache = {}
LAST_RESULTS = None  # BassKernelResults of the most recent run (for profiling)
EMIT_LOG = []  # (instruction-id watermark, label) pairs for trace analysis


def _analyze_mask(mask):
    """Classify each [128 k, 512 sq] block of mask^T. Returns (plan, dense).

    plan[i] = tuple of (j, mode, param, off) for sq-tile i; mode 0 = no
    mask, 1 = causal-triangle boundary (masked cells live in columns
    [off, off+128) and satisfy s_rel < p), 2 = dense additive mask
    (param = index into dense blocks). ``off`` is the first live sq
    column of the block (relative to the 512-wide tile); scores/exp/PV
    are restricted to [off, 512). Fully-masked blocks are omitted.
    """
    maskT = np.ascontiguousarray(mask.T)
    plan = []
    dense = []
    p_idx = np.arange(128)[:, None]
    s_idx = np.arange(512)[None, :]
    for i in range(_S // 512):
        row = []
        for j in range(_S // 128):
            blk = maskT[j * 128:(j + 1) * 128, i * 512:(i + 1) * 512]
            nz = blk != 0.0
            if nz.all():
                continue  # fully masked: block contributes nothing
            if not nz.any():
                row.append((j, 0, 0, 0))
                continue
            base = i * 512 - j * 128
            causal = (s_idx + i * 512) < (p_idx + j * 128)
            if np.array_equal(nz, causal) and np.all(blk[nz] == 1.0):
                off = max(0, -base)
                row.append((j, 1, base, off))
            else:
                row.append((j, 2, len(dense), 0))
                dense.append(blk * np.float32(_NEG))
        if row:
            # first block must start at column 0 so the PSUM accumulator
            # is fully initialised by the start=True matmul
            j0, m0, p0, _ = row[0]
            row[0] = (j0, m0, p0, 0)
        plan.append(tuple(row))
    if dense:
        dense_np = np.stack(dense).astype(np.float32)
    else:
        dense_np = np.zeros((1, 128, 512), np.float32)
    return tuple(plan), dense_np


def _build_program(plan, nblk):
    import concourse.bass as bass  # noqa: F401  (registers engine classes)
    import concourse.tile as tile
    from concourse import bacc, mybir

    F32 = mybir.dt.float32
    BF16 = mybir.dt.bfloat16
    AF = mybir.ActivationFunctionType
    ALU = mybir.AluOpType
    ts = bass.ts

    nc = bacc.Bacc(None, target_bir_lowering=False, debug=False)

    xq = nc.dram_tensor("xq", [_D, _S], BF16, kind="ExternalInput").ap()
    xk = nc.dram_tensor("xk", [_D, _S], BF16, kind="ExternalInput").ap()
    xv = nc.dram_tensor("xv", [_D, _S], BF16, kind="ExternalInput").ap()
    wq = nc.dram_tensor("wq", [_D, _DPC], BF16, kind="ExternalInput").ap()
    wk = nc.dram_tensor("wk", [_D, _DPC], BF16, kind="ExternalInput").ap()
    wv = nc.dram_tensor("wv", [_D, _DPC], BF16, kind="ExternalInput").ap()
    wo = nc.dram_tensor("wo", [_DPC, _D], BF16, kind="ExternalInput").ap()
    bq = nc.dram_tensor("bq", [_DPC], F32, kind="ExternalInput").ap()
    bk = nc.dram_tensor("bk", [_DPC], F32, kind="ExternalInput").ap()
    bvb = nc.dram_tensor("bvb", [128, _DPC], F32, kind="ExternalInput").ap()
    mblk = nc.dram_tensor("mblk", [nblk, 128, 512], F32, kind="ExternalInput").ap()
    y = nc.dram_tensor("y", [_D, _S], BF16, kind="ExternalOutput").ap()

    has_dense = any(m == 2 for row in plan for (_, m, _, _) in row)

    EMIT_LOG.clear()

    def mark(lbl):
        EMIT_LOG.append((nc.next_id(), lbl))

    with tile.TileContext(nc) as tc:
        from contextlib import ExitStack
        with ExitStack() as ctx:
            wpool = ctx.enter_context(tc.tile_pool(name="w", bufs=1))
            cpool = ctx.enter_context(tc.tile_pool(name="const", bufs=1))
            xpool = ctx.enter_context(tc.tile_pool(name="xcol", bufs=12))
            biga = ctx.enter_context(tc.tile_pool(name="biga", bufs=1))
            probp = ctx.enter_context(tc.tile_pool(name="probs", bufs=7))
            asbp = ctx.enter_context(tc.tile_pool(name="asb", bufs=4))
            bcp = ctx.enter_context(tc.tile_pool(name="bc", bufs=2))
            yp = ctx.enter_context(tc.tile_pool(name="y", bufs=4))
            mpool = (
                ctx.enter_context(tc.tile_pool(name="mstream", bufs=2))
                if has_dense else None
            )
            mmps = ctx.enter_context(tc.tile_pool(name="mmps", bufs=2, space="PSUM"))
            spsp = ctx.enter_context(tc.tile_pool(name="sps", bufs=2, space="PSUM"))
            accp = ctx.enter_context(tc.tile_pool(name="acc", bufs=2, space="PSUM"))

            xq_r = xq.rearrange("(m p) s -> p m s", p=128)
            xk_r = xk.rearrange("(m p) s -> p m s", p=128)
            xv_r = xv.rearrange("(m p) s -> p m s", p=128)
            wq_r = wq.rearrange("(m p) d -> p m d", p=128)
            wk_r = wk.rearrange("(m p) d -> p m d", p=128)
            wv_r = wv.rearrange("(m p) d -> p m d", p=128)

            # --- all input DMAs issued up front; column 0's q tensors,
            # then the (tiny) biases, then k/v, then the rest.
            bq_sb = cpool.tile([128, 2], F32, tag="bq")
            bk_sb = cpool.tile([128, 2], F32, tag="bk")
            wq_sb = wpool.tile([128, 8, _DPC], BF16, tag="wq")
            wk_sb = wpool.tile([128, 8, _DPC], BF16, tag="wk")
            wv_sb = wpool.tile([128, 8, _DPC], BF16, tag="wv")
            xq_t = [xpool.tile([128, 8, 512], BF16, tag="xcol",
                               name=f"xq_t{c}") for c in range(4)]
            xk_t = [xpool.tile([128, 8, 512], BF16, tag="xcol",
                               name=f"xk_t{c}") for c in range(4)]
            xv_t = [xpool.tile([128, 8, 512], BF16, tag="xcol",
                               name=f"xv_t{c}") for c in range(4)]
            bvb_sb = cpool.tile([128, _DPC], F32, tag="bvb")
            for w_sb, w_r, x_t, x_r in (
                (wq_sb, wq_r, xq_t[0], xq_r),
                (wk_sb, wk_r, xk_t[0], xk_r),
                (wv_sb, wv_r, xv_t[0], xv_r),
            ):
                for lo, hi in ((0, 4), (4, 8)):
                    nc.sync.dma_start(out=w_sb[:, lo:hi, :],
                                      in_=w_r[:, lo:hi, :])
                    nc.sync.dma_start(out=x_t[:, lo:hi, :],
                                      in_=x_r[:, lo:hi, ts(0, 512)])
                if w_sb is wq_sb:
                    nc.sync.dma_start(
                        out=bq_sb, in_=bq.rearrange("(h p) -> p h", p=128))
                    nc.sync.dma_start(
                        out=bk_sb, in_=bk.rearrange("(h p) -> p h", p=128))
                if w_sb is wv_sb:
                    nc.sync.dma_start(out=bvb_sb, in_=bvb)

            wo_sb = wpool.tile([128, 2, _D], BF16, tag="wo")

            # remaining x columns (prefetch, in processing order); wo is
            # only needed at the first out-projection (~45us)
            for c in (1, 3, 2):
                nc.sync.dma_start(out=xq_t[c], in_=xq_r[:, :, ts(c, 512)])
                nc.sync.dma_start(out=xk_t[c], in_=xk_r[:, :, ts(c, 512)])
                nc.sync.dma_start(out=xv_t[c], in_=xv_r[:, :, ts(c, 512)])
                if c == 1:
                    nc.sync.dma_start(
                        out=wo_sb, in_=wo.rearrange("(c p) o -> p c o", p=128))

            resident_mask = has_dense and nblk <= 2
            if resident_mask:
                mask_sb = cpool.tile([128, nblk, 512], F32, tag="mask")
                nc.sync.dma_start(
                    out=mask_sb, in_=mblk.rearrange("n p s -> p n s")
                )

            # shared triangular boundary mask: keep cell (p, s_rel) iff
            # s_rel >= p (after live-range shift every causal diagonal
            # block reduces to this)
            use_m01 = any(m == 1 for row in plan for (_, m, _, _) in row)
            if use_m01:
                m01 = cpool.tile([128, 128], BF16, tag="m01")
                nc.vector.memset(m01, 1.0)
                nc.gpsimd.affine_select(
                    out=m01, in_=m01,
                    compare_op=ALU.is_ge, fill=0.0, base=0,
                    channel_multiplier=-1, pattern=[[1, 128]],
                )

            # --- big SBUF state ---
            qT = biga.tile([128, 2, _S], BF16, tag="qT")
            kT = biga.tile([128, 2, _S], BF16, tag="kT")
            vsb = biga.tile([128, 16, _HPC * 65], BF16, tag="v")
            attn = biga.tile([128, 2, _S], BF16, tag="attn")

            # ones columns of v (softmax denominator trick): one strided
            # memset over all 16 x 4 ones columns
            vsb_ones = vsb.rearrange("p s (h x) -> p s h x", x=65)[:, :, :, 64:65]
            nc.vector.memset(vsb_ones, 1.0)

            # v-projection emitted lazily per 512-wide k-column group, the
            # first time any PV needs a chunk from it
            v_pending = set(range(4))

            def ensure_vgroup(col):
                if col not in v_pending:
                    return
                v_pending.discard(col)
                mark(f"vproj{col}")
                for c in range(4):
                    vps = mmps.tile([128, 512], F32, tag="mm", name="vps")
                    for m in range(8):
                        nc.tensor.matmul(
                            vps[:, 0:_DPC], lhsT=xv_t[col][:, m, ts(c, 128)],
                            rhs=wv_sb[:, m, :], start=(m == 0), stop=(m == 7),
                        )
                    sc = col * 4 + c
                    nc.vector.tensor_add(
                        vsb[:, sc, 0:260].rearrange(
                            "p (h x) -> p h x", x=65)[:, :, 0:64],
                        vps[:, 0:_DPC].rearrange("p (h x) -> p h x", x=64),
                        bvb_sb.rearrange("p (h x) -> p h x", x=64),
                    )

            def proj_unit(which, st, dh):
                # one q- or k-projection unit: 8 accumulating matmuls
                # (contraction over D) + DVE bias add into qT/kT
                mark(f"proj_{which}{st}d{dh}")
                x_t, w_sb, b_sb, dst = (
                    (xq_t[st], wq_sb, bq_sb, qT) if which == "q"
                    else (xk_t[st], wk_sb, bk_sb, kT)
                )
                ps = mmps.tile([128, 512], F32, tag="mm", name=f"{which}ps")
                for m in range(8):
                    nc.tensor.matmul(
                        ps, lhsT=w_sb[:, m, ts(dh, 128)], rhs=x_t[:, m, :],
                        start=(m == 0), stop=(m == 7),
                    )
                nc.vector.tensor_scalar(
                    dst[:, dh, ts(st, 512)], ps, b_sb[:, dh:dh + 1], None,
                    ALU.add,
                )

            # deferred work units (FIFO), pumped between attention blocks
            # so the PE always has ready work while exp latency drains.
            # ``reserve`` units are held for the very end of the kernel to
            # fill the PE while the last column's softmax chain drains.
            fillers = []   # entries: (proj_col_or_None, emit_fn)
            reserve = []

            def pump(n=1):
                for _ in range(min(n, len(fillers))):
                    fillers.pop(0)[1]()

            def flush_proj_upto(col):
                # correctness: Tile tracks dependencies in emission order,
                # so any projection for a column whose kT/qT this column's
                # attention reads must be emitted before the blocks
                keep = []
                for ent in fillers:
                    if ent[0] is not None and ent[0] <= col:
                        ent[1]()
                    else:
                        keep.append(ent)
                fillers[:] = keep

            def norm_finish(acc_sb, rec, g, i):
                # broadcast the reciprocal across partitions on GpSimd,
                # then normalise into attn (all-SBUF DVE muls)
                mark(f"normfin_g{g}_c{i}")
                bc_sb = bcp.tile([64, 2, 512], F32, tag="bc", name="bc_sb")
                nc.gpsimd.partition_broadcast(bc_sb, rec, channels=64)
                for hh in range(2):
                    nc.vector.tensor_mul(
                        attn[hh * 64:(hh + 1) * 64, g, ts(i, 512)],
                        acc_sb[0:64, hh, :], bc_sb[:, hh, :],
                    )

            def fast_norm_panel(acc, g, i, lo, hi):
                # per-hh reciprocal/broadcast/multiply for sq columns
                # [lo, hi) straight off the PSUM accumulator
                w = hi - lo
                recl = [None, None]
                for hh in range(2):
                    recl[hh] = bcp.tile([1, 512], F32, tag="recl",
                                        name="recl", bufs=2)
                    with nc.allow_low_precision(
                        reason="softmax reciprocal"
                    ):
                        nc.vector.reciprocal(
                            recl[hh][:, 0:w], acc[hh][64:65, lo:hi])
                bcl = [None, None]
                for hh in range(2):
                    bcl[hh] = bcp.tile([64, 512], F32, tag="bcl",
                                       name="bcl", bufs=2)
                    nc.gpsimd.partition_broadcast(
                        bcl[hh][:, 0:w], recl[hh][:, 0:w], channels=64)
                for hh in range(2):
                    nc.vector.tensor_mul(
                        attn[hh * 64:(hh + 1) * 64, g,
                             i * 512 + lo:i * 512 + hi],
                        acc[hh][0:64, lo:hi], bcl[hh][:, 0:w],
                    )

            def outproj_unit_last(st, oc, pool, act_copy):
                # final-column out-projection, split by sq panel into two
                # independent half-bank PSUM tiles so each half's matmuls,
                # PSUM->SBUF copy and store start as soon as its slice of
                # attn is normalised
                mark(f"outproj{st}_oc{oc}")
                tag = "mm" if pool is mmps else "sps"
                y_sb = yp.tile([128, 512], BF16, tag="y", name="y_sb")
                for lo, hi in ((0, 256), (256, 512)):
                    yps = pool.tile([128, 256], F32, tag=tag, name="yps")
                    for cc in range(2):
                        nc.tensor.matmul(
                            yps, lhsT=wo_sb[:, cc, ts(oc, 128)],
                            rhs=attn[:, cc, st * 512 + lo:st * 512 + hi],
                            start=(cc == 0), stop=(cc == 1),
                        )
                    if act_copy:
                        nc.scalar.copy(y_sb[:, lo:hi], yps)
                    else:
                        nc.vector.tensor_copy(y_sb[:, lo:hi], yps)
                nc.sync.dma_start(
                    out=y[oc * 128:(oc + 1) * 128, ts(st, 512)], in_=y_sb
                )

            def outproj_unit(st, oc, pool=None, act_copy=False,
                             pool_dma=False):
                # output projection for one 128-row slice of y (partial)
                mark(f"outproj{st}_oc{oc}")
                pool = pool or mmps
                tag = "mm" if pool is mmps else "sps"
                yps = pool.tile([128, 512], F32, tag=tag, name="yps")
                for cc in range(2):
                    nc.tensor.matmul(
                        yps, lhsT=wo_sb[:, cc, ts(oc, 128)],
                        rhs=attn[:, cc, ts(st, 512)],
                        start=(cc == 0), stop=(cc == 1),
                    )
                y_sb = yp.tile([128, 512], BF16, tag="y", name="y_sb")
                if act_copy:
                    nc.scalar.copy(y_sb, yps)
                else:
                    nc.vector.tensor_copy(y_sb, yps)
                eng = nc.gpsimd if pool_dma else nc.sync
                eng.dma_start(
                    out=y[oc * 128:(oc + 1) * 128, ts(st, 512)], in_=y_sb
                )

            # --- fused pipeline over sq columns. Processing order puts
            # the largest column (3) third so the deferred work of its
            # predecessor fills its exp-paced bubbles, and column 2 last
            # (its own bubbles absorb column 3's deferred norm/outproj).
            cols_order = (0, 1, 3, 2)
            for idx, st in enumerate(cols_order):
                nxt = cols_order[idx + 1] if idx + 1 < len(cols_order) else None
                last = nxt is None
                if idx == 0:
                    proj_unit("q", st, 0)
                    proj_unit("k", st, 0)
                    fillers.append((0, lambda: proj_unit("q", 0, 1)))
                    fillers.append((0, lambda: proj_unit("k", 0, 1)))
                # queue all remaining columns' projections as filler work
                if idx == 0:
                    for qc in cols_order[1:]:
                        for dh in range(2):
                            fillers.append(
                                (qc,
                                 lambda dh=dh, c=qc: proj_unit("q", c, dh)))
                            fillers.append(
                                (qc,
                                 lambda dh=dh, c=qc: proj_unit("k", c, dh)))
                # correctness: every kT column this column's blocks read
                # must be projected in emission order first
                need = max((b[0] // 4 for b in plan[st]), default=0)
                flush_proj_upto(max(st, need))

                i = st
                blocks = plan[i]
                nj = len(blocks)
                for g in range(2):
                    # two independent 1-bank accumulators (one per hh) so
                    # the next group's P@V unblocks per-half as each copy
                    # drains, instead of waiting for the full 2-bank tile
                    acc = [accp.tile([65, 512], F32, tag="acc",
                                     name=f"acc{st}{g}h{hh}")
                           for hh in range(2)]

                    def emit_pv(pend, start, stop):
                        j_p, off_p, probs_p = pend
                        for hh in range(2):
                            h = 2 * g + hh
                            nc.tensor.matmul(
                                acc[hh][:, off_p:512],
                                lhsT=vsb[:, j_p, h * 65:(h + 1) * 65],
                                rhs=probs_p[:, hh, off_p:512],
                                start=start, stop=stop,
                            )

                    tail_g = last and g == 1
                    if tail_g:
                        # flush deferred work now so its DVE traffic lands
                        # ahead of the softmax-chain ops in the DVE queue
                        while fillers:
                            pump(1)
                        for r in reserve:
                            r()
                    pend = None  # scores/exp run one block ahead of PV
                    for bi, (j, mode, param, off) in enumerate(blocks):
                        mark(f"blk_c{st}g{g}j{j}")
                        ensure_vgroup(j // 4)
                        sps = spsp.tile([128, 2, 512], F32, tag="sps", name="sps")
                        for hh in range(2):
                            nc.tensor.matmul(
                                sps[:, hh, off:512],
                                lhsT=kT[hh * 64:(hh + 1) * 64, g, ts(j, 128)],
                                rhs=qT[hh * 64:(hh + 1) * 64, g,
                                       i * 512 + off:(i + 1) * 512],
                                start=True, stop=True,
                            )
                        if mode == 2:
                            if resident_mask:
                                mt = mask_sb[:, param, :]
                            else:
                                mt = mpool.tile([128, 512], F32, tag="mtile",
                                                name="mt")
                                nc.sync.dma_start(out=mt, in_=mblk[param])
                            for hh in range(2):
                                nc.vector.tensor_add(
                                    sps[:, hh, :], sps[:, hh, :], mt
                                )
                        probs = probp.tile([128, 2, 512], BF16, tag="probs",
                                           name="probs")
                        nc.scalar.activation(
                            probs[:, :, off:512], sps[:, :, off:512], AF.Exp
                        )
                        if mode == 1:
                            # masked cells sit in columns [off, off+128):
                            # s_rel < p relative to the live window
                            for hh in range(2):
                                nc.vector.tensor_mul(
                                    probs[:, hh, off:off + 128],
                                    probs[:, hh, off:off + 128],
                                    m01,
                                )
                        if pend is not None:
                            emit_pv(pend, start=(bi == 1), stop=False)
                            pump(1)
                        pend = (j, off, probs)
                    emit_pv(pend, start=(nj == 1), stop=True)
                    if tail_g:
                        fast_norm_panel(acc, g, i, 0, 512)
                    pump(2)

                    if not tail_g:
                        # copy the accumulator out of PSUM promptly (frees
                        # the 2-bank slot for the other head-pair group) and
                        # take the reciprocal; the rest of the normalisation
                        # is deferred as filler work
                        mark(f"acccopy_c{st}g{g}")
                        acc_sb = asbp.tile([65, 2, 512], F32, tag="asb",
                                           name="acc_sb")
                        for hh in range(2):
                            nc.vector.tensor_copy(acc_sb[:, hh, :], acc[hh])
                        rec = bcp.tile([1, 2, 512], F32, tag="rec",
                                       name="rec", bufs=3)
                        with nc.allow_low_precision(
                            reason="softmax reciprocal"
                        ):
                            nc.vector.reciprocal(rec, acc_sb[64:65, :, :])
                        if not last:
                            fillers.append(
                                (None, lambda a=acc_sb, r=rec, g=g, i=i:
                                 norm_finish(a, r, g, i)))
                            if g == 1:
                                for oc in range(8):
                                    item = (None,
                                            lambda st=st, oc=oc:
                                            outproj_unit(st, oc))
                                    if (idx == len(cols_order) - 2
                                            and oc >= 4):
                                        reserve.append(item[1])
                                    else:
                                        fillers.append(item)
                        else:
                            norm_finish(acc_sb, rec, g, i)

                if last:
                    for oc in range(8):
                        outproj_unit(st, oc,
                                     pool=(spsp if oc % 2 else mmps),
                                     act_copy=bool(oc % 2))

            while fillers:
                pump(1)

    nc.compile()
    return nc


def kernel(**inputs):
    global LAST_RESULTS
    from concourse.bass_utils import run_bass_kernel_spmd

    Q = np.asarray(inputs["Q"], dtype=np.float32)
    K = np.asarray(inputs["K"], dtype=np.float32)
    V = np.asarray(inputs["V"], dtype=np.float32)
    mask = np.asarray(inputs["mask"], dtype=np.float32)
    Wq = np.asarray(inputs["Wq"], dtype=np.float32)
    bq = np.asarray(inputs["bq"], dtype=np.float32)
    Wk = np.asarray(inputs["Wk"], dtype=np.float32)
    bk = np.asarray(inputs["bk"], dtype=np.float32)
    Wv = np.asarray(inputs["Wv"], dtype=np.float32)
    bv = np.asarray(inputs["bv"], dtype=np.float32)
    Wo = np.asarray(inputs["Wo"], dtype=np.float32)
    bo = np.asarray(inputs["bo"], dtype=np.float32)

    plan, dense = _analyze_mask(mask)
    key = (plan, dense.shape[0])
    if key not in _program_cache:
        _program_cache[key] = _build_program(plan, dense.shape[0])
    nc = _program_cache[key]

    import ml_dtypes
    bf16 = ml_dtypes.bfloat16
    sc = np.float32(1.0 / np.sqrt(_DK))
    xqT = [np.ascontiguousarray(Q[b].T).astype(bf16) for b in range(_B)]
    xkT = [np.ascontiguousarray(K[b].T).astype(bf16) for b in range(_B)]
    xvT = [np.ascontiguousarray(V[b].T).astype(bf16) for b in range(_B)]

    in_maps = []
    for core in range(_NCORES):
        b = core // _CPG
        rows = slice((core % _CPG) * _DPC, (core % _CPG) * _DPC + _DPC)
        in_maps.append({
            "xq": xqT[b], "xk": xkT[b], "xv": xvT[b],
            "wq": np.ascontiguousarray((Wq[rows] * sc).T).astype(bf16),
            "wk": np.ascontiguousarray(Wk[rows].T).astype(bf16),
            "wv": np.ascontiguousarray(Wv[rows].T).astype(bf16),
            "wo": np.ascontiguousarray(Wo[:, rows].T).astype(bf16),
            "bq": np.ascontiguousarray(bq[rows] * sc),
            "bk": np.ascontiguousarray(bk[rows]),
            "bvb": np.broadcast_to(bv[rows], (128, _DPC)).copy(),
            "mblk": dense,
        })

    trace = bool(int(os.environ.get("KERNEL_TRACE", "0")))
    LAST_RESULTS = run_bass_kernel_spmd(
        nc, in_maps, list(range(_NCORES)), trace=trace
    )

    out = np.empty((_B, _S, _D), np.float32)
    for b in range(_B):
        acc = np.zeros((_D, _S), np.float32)
        for c in range(_CPG):
            acc += LAST_RESULTS.results[b * _CPG + c]["y"].astype(np.float32)
        out[b] = (acc.T + bo).astype(np.float32)
    return out


# revision 33
# speedup vs baseline: 1.2324x; 1.0033x over previous
"""Multi-head attention (B=2, S=2048, D=1024, H=16) on 8 TRN2 NeuronCores.

Sharding: data-parallel over batch (2 groups of 4 cores) x head-parallel
(4 heads per core). W_q/W_k/W_v are column-sharded by head, W_o is
row-sharded; the 4 partial W_o outputs per batch are summed on the host
(the unshard step), which also undoes the device-side transposed layout.

Per-core kernel design (bf16 compute, fp32 PSUM accumulation):
  - All tensor operands (x, W_q/k/v/o, qT/kT/v, probs, attn, y) are bf16;
    PSUM accumulates in fp32, softmax denominators/normalisation in fp32.
    End-to-end relative error ~6e-3 (gate is 2e-2).
  - Host pre-transposes X and the weight slices so every matmul
    contraction sits on the partition dim; 1/sqrt(d_k) folded into W_q.
  - q/k projections produce qT/kT in [128 = 2 heads x 64 d, S] layout;
    v in natural [S, d] layout with a ones column per head so the P@V
    matmul accumulates the softmax denominator for free.
  - scores are computed transposed ([k, sq] blocks); softmax skips the
    max-subtraction (scores are O(5), exp accumulated in fp32).
  - causal structure: fully-masked [128 k x 512 sq] blocks are skipped;
    diagonal blocks are truncated to their live column range (width
    512/384/256/128) for scores, exp and P@V; the remaining triangular
    boundary is zeroed with a single shared [128,128] 0/1 mask.
  - denominator reciprocal is broadcast across partitions on the (idle)
    GpSimd engine; the per-g PSUM accumulator is copied to SBUF as soon
    as P@V finishes so the 2-bank PSUM slot frees early.
  - software-pipelined emission: each column's normalisation finish and
    output projection are deferred one column and interleaved as filler
    PE work between attention blocks (which are ACT-exp paced), so the
    PE's in-order queue never head-of-line blocks on the softmax chain.
  - all input DMAs are issued up front so the SP DMA queue never blocks
    input prefetch behind compute-dependent output stores.
"""

import os

import numpy as np

_B, _S, _D, _H, _DK = 2, 2048, 1024, 16, 64
_HPC = 4          # heads per core
_NCORES = 8
_CPG = 4          # cores per (batch) group
_DPC = _HPC * _DK # 256 projection dims per core
_NEG = -1e9

_program_cache = {}
LAST_RESULTS = None  # BassKernelResults of the most recent run (for profiling)
EMIT_LOG = []  # (instruction-id watermark, label) pairs for trace analysis


def _analyze_mask(mask):
    """Classify each [128 k, 512 sq] block of mask^T. Returns (plan, dense).

    plan[i] = tuple of (j, mode, param, off) for sq-tile i; mode 0 = no
    mask, 1 = causal-triangle boundary (masked cells live in columns
    [off, off+128) and satisfy s_rel < p), 2 = dense additive mask
    (param = index into dense blocks). ``off`` is the first live sq
    column of the block (relative to the 512-wide tile); scores/exp/PV
    are restricted to [off, 512). Fully-masked blocks are omitted.
    """
    maskT = np.ascontiguousarray(mask.T)
    plan = []
    dense = []
    p_idx = np.arange(128)[:, None]
    s_idx = np.arange(512)[None, :]
    for i in range(_S // 512):
        row = []
        for j in range(_S // 128):
            blk = maskT[j * 128:(j + 1) * 128, i * 512:(i + 1) * 512]
            nz = blk != 0.0
            if nz.all():
                continue  # fully masked: block contributes nothing
            if not nz.any():
                row.append((j, 0, 0, 0))
                continue
            base = i * 512 - j * 128
            causal = (s_idx + i * 512) < (p_idx + j * 128)
            if np.array_equal(nz, causal) and np.all(blk[nz] == 1.0):
                off = max(0, -base)
                row.append((j, 1, base, off))
            else:
                row.append((j, 2, len(dense), 0))
                dense.append(blk * np.float32(_NEG))
        if row:
            # first block must start at column 0 so the PSUM accumulator
            # is fully initialised by the start=True matmul
            j0, m0, p0, _ = row[0]
            row[0] = (j0, m0, p0, 0)
        plan.append(tuple(row))
    if dense:
        dense_np = np.stack(dense).astype(np.float32)
    else:
        dense_np = np.zeros((1, 128, 512), np.float32)
    return tuple(plan), dense_np


def _build_program(plan, nblk):
    import concourse.bass as bass  # noqa: F401  (registers engine classes)
    import concourse.tile as tile
    from concourse import bacc, mybir

    F32 = mybir.dt.float32
    BF16 = mybir.dt.bfloat16
    AF = mybir.ActivationFunctionType
    ALU = mybir.AluOpType
    ts = bass.ts

    nc = bacc.Bacc(None, target_bir_lowering=False, debug=False)

    xq = nc.dram_tensor("xq", [_D, _S], BF16, kind="ExternalInput").ap()
    xk = nc.dram_tensor("xk", [_D, _S], BF16, kind="ExternalInput").ap()
    xv = nc.dram_tensor("xv", [_D, _S], BF16, kind="ExternalInput").ap()
    wq = nc.dram_tensor("wq", [_D, _DPC], BF16, kind="ExternalInput").ap()
    wk = nc.dram_tensor("wk", [_D, _DPC], BF16, kind="ExternalInput").ap()
    wv = nc.dram_tensor("wv", [_D, _DPC], BF16, kind="ExternalInput").ap()
    wo = nc.dram_tensor("wo", [_DPC, _D], BF16, kind="ExternalInput").ap()
    bq = nc.dram_tensor("bq", [_DPC], F32, kind="ExternalInput").ap()
    bk = nc.dram_tensor("bk", [_DPC], F32, kind="ExternalInput").ap()
    bvb = nc.dram_tensor("bvb", [128, _DPC], F32, kind="ExternalInput").ap()
    mblk = nc.dram_tensor("mblk", [nblk, 128, 512], F32, kind="ExternalInput").ap()
    y = nc.dram_tensor("y", [_D, _S], BF16, kind="ExternalOutput").ap()

    has_dense = any(m == 2 for row in plan for (_, m, _, _) in row)

    EMIT_LOG.clear()

    def mark(lbl):
        EMIT_LOG.append((nc.next_id(), lbl))

    with tile.TileContext(nc) as tc:
        from contextlib import ExitStack
        with ExitStack() as ctx:
            wpool = ctx.enter_context(tc.tile_pool(name="w", bufs=1))
            cpool = ctx.enter_context(tc.tile_pool(name="const", bufs=1))
            xpool = ctx.enter_context(tc.tile_pool(name="xcol", bufs=12))
            biga = ctx.enter_context(tc.tile_pool(name="biga", bufs=1))
            probp = ctx.enter_context(tc.tile_pool(name="probs", bufs=7))
            asbp = ctx.enter_context(tc.tile_pool(name="asb", bufs=4))
            bcp = ctx.enter_context(tc.tile_pool(name="bc", bufs=2))
            yp = ctx.enter_context(tc.tile_pool(name="y", bufs=4))
            mpool = (
                ctx.enter_context(tc.tile_pool(name="mstream", bufs=2))
                if has_dense else None
            )
            mmps = ctx.enter_context(tc.tile_pool(name="mmps", bufs=2, space="PSUM"))
            spsp = ctx.enter_context(tc.tile_pool(name="sps", bufs=2, space="PSUM"))
            accp = ctx.enter_context(tc.tile_pool(name="acc", bufs=2, space="PSUM"))

            xq_r = xq.rearrange("(m p) s -> p m s", p=128)
            xk_r = xk.rearrange("(m p) s -> p m s", p=128)
            xv_r = xv.rearrange("(m p) s -> p m s", p=128)
            wq_r = wq.rearrange("(m p) d -> p m d", p=128)
            wk_r = wk.rearrange("(m p) d -> p m d", p=128)
            wv_r = wv.rearrange("(m p) d -> p m d", p=128)

            # --- all input DMAs issued up front; column 0's q tensors,
            # then the (tiny) biases, then k/v, then the rest.
            bq_sb = cpool.tile([128, 2], F32, tag="bq")
            bk_sb = cpool.tile([128, 2], F32, tag="bk")
            wq_sb = wpool.tile([128, 8, _DPC], BF16, tag="wq")
            wk_sb = wpool.tile([128, 8, _DPC], BF16, tag="wk")
            wv_sb = wpool.tile([128, 8, _DPC], BF16, tag="wv")
            xq_t = [xpool.tile([128, 8, 512], BF16, tag="xcol",
                               name=f"xq_t{c}") for c in range(4)]
            xk_t = [xpool.tile([128, 8, 512], BF16, tag="xcol",
                               name=f"xk_t{c}") for c in range(4)]
            xv_t = [xpool.tile([128, 8, 512], BF16, tag="xcol",
                               name=f"xv_t{c}") for c in range(4)]
            bvb_sb = cpool.tile([128, _DPC], F32, tag="bvb")
            for w_sb, w_r, x_t, x_r in (
                (wq_sb, wq_r, xq_t[0], xq_r),
                (wk_sb, wk_r, xk_t[0], xk_r),
                (wv_sb, wv_r, xv_t[0], xv_r),
            ):
                for lo, hi in ((0, 4), (4, 8)):
                    nc.sync.dma_start(out=w_sb[:, lo:hi, :],
                                      in_=w_r[:, lo:hi, :])
                    nc.sync.dma_start(out=x_t[:, lo:hi, :],
                                      in_=x_r[:, lo:hi, ts(0, 512)])
                if w_sb is wq_sb:
                    nc.sync.dma_start(
                        out=bq_sb, in_=bq.rearrange("(h p) -> p h", p=128))
                    nc.sync.dma_start(
                        out=bk_sb, in_=bk.rearrange("(h p) -> p h", p=128))
                if w_sb is wv_sb:
                    nc.sync.dma_start(out=bvb_sb, in_=bvb)

            wo_sb = wpool.tile([128, 2, _D], BF16, tag="wo")

            # remaining x columns (prefetch, in processing order); wo is
            # only needed at the first out-projection (~45us)
            for c in (1, 3, 2):
                nc.sync.dma_start(out=xq_t[c], in_=xq_r[:, :, ts(c, 512)])
                nc.sync.dma_start(out=xk_t[c], in_=xk_r[:, :, ts(c, 512)])
                nc.sync.dma_start(out=xv_t[c], in_=xv_r[:, :, ts(c, 512)])
                if c == 1:
                    nc.sync.dma_start(
                        out=wo_sb, in_=wo.rearrange("(c p) o -> p c o", p=128))

            resident_mask = has_dense and nblk <= 2
            if resident_mask:
                mask_sb = cpool.tile([128, nblk, 512], F32, tag="mask")
                nc.sync.dma_start(
                    out=mask_sb, in_=mblk.rearrange("n p s -> p n s")
                )

            # shared triangular boundary mask: keep cell (p, s_rel) iff
            # s_rel >= p (after live-range shift every causal diagonal
            # block reduces to this)
            use_m01 = any(m == 1 for row in plan for (_, m, _, _) in row)
            if use_m01:
                m01 = cpool.tile([128, 128], BF16, tag="m01")
                nc.vector.memset(m01, 1.0)
                nc.gpsimd.affine_select(
                    out=m01, in_=m01,
                    compare_op=ALU.is_ge, fill=0.0, base=0,
                    channel_multiplier=-1, pattern=[[1, 128]],
                )

            # --- big SBUF state ---
            qT = biga.tile([128, 2, _S], BF16, tag="qT")
            kT = biga.tile([128, 2, _S], BF16, tag="kT")
            vsb = biga.tile([128, 16, _HPC * 65], BF16, tag="v")
            attn = biga.tile([128, 2, _S], BF16, tag="attn")

            # ones columns of v (softmax denominator trick): one strided
            # memset over all 16 x 4 ones columns
            vsb_ones = vsb.rearrange("p s (h x) -> p s h x", x=65)[:, :, :, 64:65]
            nc.vector.memset(vsb_ones, 1.0)

            # v-projection emitted lazily per 512-wide k-column group, the
            # first time any PV needs a chunk from it
            v_pending = set(range(4))

            def ensure_vgroup(col):
                if col not in v_pending:
                    return
                v_pending.discard(col)
                mark(f"vproj{col}")
                for c in range(4):
                    vps = mmps.tile([128, 512], F32, tag="mm", name="vps")
                    for m in range(8):
                        nc.tensor.matmul(
                            vps[:, 0:_DPC], lhsT=xv_t[col][:, m, ts(c, 128)],
                            rhs=wv_sb[:, m, :], start=(m == 0), stop=(m == 7),
                        )
                    sc = col * 4 + c
                    nc.vector.tensor_add(
                        vsb[:, sc, 0:260].rearrange(
                            "p (h x) -> p h x", x=65)[:, :, 0:64],
                        vps[:, 0:_DPC].rearrange("p (h x) -> p h x", x=64),
                        bvb_sb.rearrange("p (h x) -> p h x", x=64),
                    )

            def proj_unit(which, st, dh):
                # one q- or k-projection unit: 8 accumulating matmuls
                # (contraction over D) + DVE bias add into qT/kT
                mark(f"proj_{which}{st}d{dh}")
                x_t, w_sb, b_sb, dst = (
                    (xq_t[st], wq_sb, bq_sb, qT) if which == "q"
                    else (xk_t[st], wk_sb, bk_sb, kT)
                )
                ps = mmps.tile([128, 512], F32, tag="mm", name=f"{which}ps")
                for m in range(8):
                    nc.tensor.matmul(
                        ps, lhsT=w_sb[:, m, ts(dh, 128)], rhs=x_t[:, m, :],
                        start=(m == 0), stop=(m == 7),
                    )
                nc.vector.tensor_scalar(
                    dst[:, dh, ts(st, 512)], ps, b_sb[:, dh:dh + 1], None,
                    ALU.add,
                )

            # deferred work units (FIFO), pumped between attention blocks
            # so the PE always has ready work while exp latency drains.
            # ``reserve`` units are held for the very end of the kernel to
            # fill the PE while the last column's softmax chain drains.
            fillers = []   # entries: (proj_col_or_None, emit_fn)
            reserve = []

            def pump(n=1):
                for _ in range(min(n, len(fillers))):
                    fillers.pop(0)[1]()

            def flush_proj_upto(col):
                # correctness: Tile tracks dependencies in emission order,
                # so any projection for a column whose kT/qT this column's
                # attention reads must be emitted before the blocks
                keep = []
                for ent in fillers:
                    if ent[0] is not None and ent[0] <= col:
                        ent[1]()
                    else:
                        keep.append(ent)
                fillers[:] = keep

            def norm_finish(acc_sb, rec, g, i):
                # broadcast the reciprocal across partitions on GpSimd,
                # then normalise into attn (all-SBUF DVE muls)
                mark(f"normfin_g{g}_c{i}")
                bc_sb = bcp.tile([64, 2, 512], F32, tag="bc", name="bc_sb")
                nc.gpsimd.partition_broadcast(bc_sb, rec, channels=64)
                for hh in range(2):
                    nc.vector.tensor_mul(
                        attn[hh * 64:(hh + 1) * 64, g, ts(i, 512)],
                        acc_sb[0:64, hh, :], bc_sb[:, hh, :],
                    )

            def fast_norm_panel(acc, g, i, lo, hi):
                # per-hh reciprocal/broadcast/multiply for sq columns
                # [lo, hi) straight off the PSUM accumulator
                w = hi - lo
                recl = [None, None]
                for hh in range(2):
                    recl[hh] = bcp.tile([1, 512], F32, tag="recl",
                                        name="recl", bufs=2)
                    with nc.allow_low_precision(
                        reason="softmax reciprocal"
                    ):
                        nc.vector.reciprocal(
                            recl[hh][:, 0:w], acc[hh][64:65, lo:hi])
                bcl = [None, None]
                for hh in range(2):
                    bcl[hh] = bcp.tile([64, 512], F32, tag="bcl",
                                       name="bcl", bufs=2)
                    nc.gpsimd.partition_broadcast(
                        bcl[hh][:, 0:w], recl[hh][:, 0:w], channels=64)
                for hh in range(2):
                    nc.vector.tensor_mul(
                        attn[hh * 64:(hh + 1) * 64, g,
                             i * 512 + lo:i * 512 + hi],
                        acc[hh][0:64, lo:hi], bcl[hh][:, 0:w],
                    )

            def outproj_unit_last(st, oc, pool, act_copy):
                # final-column out-projection, split by sq panel into two
                # independent half-bank PSUM tiles so each half's matmuls,
                # PSUM->SBUF copy and store start as soon as its slice of
                # attn is normalised
                mark(f"outproj{st}_oc{oc}")
                tag = "mm" if pool is mmps else "sps"
                y_sb = yp.tile([128, 512], BF16, tag="y", name="y_sb")
                for lo, hi in ((0, 256), (256, 512)):
                    yps = pool.tile([128, 256], F32, tag=tag, name="yps")
                    for cc in range(2):
                        nc.tensor.matmul(
                            yps, lhsT=wo_sb[:, cc, ts(oc, 128)],
                            rhs=attn[:, cc, st * 512 + lo:st * 512 + hi],
                            start=(cc == 0), stop=(cc == 1),
                        )
                    if act_copy:
                        nc.scalar.copy(y_sb[:, lo:hi], yps)
                    else:
                        nc.vector.tensor_copy(y_sb[:, lo:hi], yps)
                nc.sync.dma_start(
                    out=y[oc * 128:(oc + 1) * 128, ts(st, 512)], in_=y_sb
                )

            def outproj_unit(st, oc, pool=None, act_copy=False,
                             pool_dma=False):
                # output projection for one 128-row slice of y (partial)
                mark(f"outproj{st}_oc{oc}")
                pool = pool or mmps
                tag = "mm" if pool is mmps else "sps"
                yps = pool.tile([128, 512], F32, tag=tag, name="yps")
                for cc in range(2):
                    nc.tensor.matmul(
                        yps, lhsT=wo_sb[:, cc, ts(oc, 128)],
                        rhs=attn[:, cc, ts(st, 512)],
                        start=(cc == 0), stop=(cc == 1),
                    )
                y_sb = yp.tile([128, 512], BF16, tag="y", name="y_sb")
                if act_copy:
                    nc.scalar.copy(y_sb, yps)
                else:
                    nc.vector.tensor_copy(y_sb, yps)
                eng = nc.gpsimd if pool_dma else nc.sync
                eng.dma_start(
                    out=y[oc * 128:(oc + 1) * 128, ts(st, 512)], in_=y_sb
                )

            # --- fused pipeline over sq columns. Processing order puts
            # the largest column (3) third so the deferred work of its
            # predecessor fills its exp-paced bubbles, and column 2 last
            # (its own bubbles absorb column 3's deferred norm/outproj).
            cols_order = (0, 1, 3, 2)
            for idx, st in enumerate(cols_order):
                nxt = cols_order[idx + 1] if idx + 1 < len(cols_order) else None
                last = nxt is None
                if idx == 0:
                    # column 0's q projection, both dh slots interleaved at
                    # half-contraction granularity: dh1's first m-chunks
                    # only need the first xq half-DMA, so they fill the PE
                    # while the second half is still in flight
                    mark("proj_q0_split")
                    qps2 = [mmps.tile([128, 512], F32, tag="mm",
                                      name=f"qps{d}") for d in range(2)]
                    for dh in range(2):
                        for m in range(4):
                            nc.tensor.matmul(
                                qps2[dh], lhsT=wq_sb[:, m, ts(dh, 128)],
                                rhs=xq_t[0][:, m, :],
                                start=(m == 0), stop=False,
                            )
                    for dh in range(2):
                        for m in range(4, 8):
                            nc.tensor.matmul(
                                qps2[dh], lhsT=wq_sb[:, m, ts(dh, 128)],
                                rhs=xq_t[0][:, m, :],
                                start=False, stop=(m == 7),
                            )
                        nc.vector.tensor_scalar(
                            qT[:, dh, ts(0, 512)], qps2[dh],
                            bq_sb[:, dh:dh + 1], None, ALU.add,
                        )
                    proj_unit("k", st, 0)
                    fillers.append((0, lambda: proj_unit("k", 0, 1)))
                # queue all remaining columns' projections as filler work
                if idx == 0:
                    for qc in cols_order[1:]:
                        for dh in range(2):
                            fillers.append(
                                (qc,
                                 lambda dh=dh, c=qc: proj_unit("q", c, dh)))
                            fillers.append(
                                (qc,
                                 lambda dh=dh, c=qc: proj_unit("k", c, dh)))
                # correctness: every kT column this column's blocks read
                # must be projected in emission order first
                need = max((b[0] // 4 for b in plan[st]), default=0)
                flush_proj_upto(max(st, need))

                i = st
                blocks = plan[i]
                nj = len(blocks)
                for g in range(2):
                    # two independent 1-bank accumulators (one per hh) so
                    # the next group's P@V unblocks per-half as each copy
                    # drains, instead of waiting for the full 2-bank tile
                    acc = [accp.tile([65, 512], F32, tag="acc",
                                     name=f"acc{st}{g}h{hh}")
                           for hh in range(2)]

                    def emit_pv(pend, start, stop):
                        j_p, off_p, probs_p = pend
                        for hh in range(2):
                            h = 2 * g + hh
                            nc.tensor.matmul(
                                acc[hh][:, off_p:512],
                                lhsT=vsb[:, j_p, h * 65:(h + 1) * 65],
                                rhs=probs_p[:, hh, off_p:512],
                                start=start, stop=stop,
                            )

                    tail_g = last and g == 1
                    if tail_g:
                        # flush deferred work now so its DVE traffic lands
                        # ahead of the softmax-chain ops in the DVE queue
                        while fillers:
                            pump(1)
                        for r in reserve:
                            r()
                    pend = None  # scores/exp run one block ahead of PV
                    for bi, (j, mode, param, off) in enumerate(blocks):
                        mark(f"blk_c{st}g{g}j{j}")
                        ensure_vgroup(j // 4)
                        sps = spsp.tile([128, 2, 512], F32, tag="sps", name="sps")
                        for hh in range(2):
                            nc.tensor.matmul(
                                sps[:, hh, off:512],
                                lhsT=kT[hh * 64:(hh + 1) * 64, g, ts(j, 128)],
                                rhs=qT[hh * 64:(hh + 1) * 64, g,
                                       i * 512 + off:(i + 1) * 512],
                                start=True, stop=True,
                            )
                        if mode == 2:
                            if resident_mask:
                                mt = mask_sb[:, param, :]
                            else:
                                mt = mpool.tile([128, 512], F32, tag="mtile",
                                                name="mt")
                                nc.sync.dma_start(out=mt, in_=mblk[param])
                            for hh in range(2):
                                nc.vector.tensor_add(
                                    sps[:, hh, :], sps[:, hh, :], mt
                                )
                        probs = probp.tile([128, 2, 512], BF16, tag="probs",
                                           name="probs")
                        nc.scalar.activation(
                            probs[:, :, off:512], sps[:, :, off:512], AF.Exp
                        )
                        if mode == 1:
                            # masked cells sit in columns [off, off+128):
                            # s_rel < p relative to the live window
                            for hh in range(2):
                                nc.vector.tensor_mul(
                                    probs[:, hh, off:off + 128],
                                    probs[:, hh, off:off + 128],
                                    m01,
                                )
                        if pend is not None:
                            emit_pv(pend, start=(bi == 1), stop=False)
                            pump(1)
                        pend = (j, off, probs)
                    emit_pv(pend, start=(nj == 1), stop=True)
                    if tail_g:
                        fast_norm_panel(acc, g, i, 0, 512)
                    pump(3)

                    if not tail_g:
                        # copy the accumulator out of PSUM promptly (frees
                        # the 2-bank slot for the other head-pair group) and
                        # take the reciprocal; the rest of the normalisation
                        # is deferred as filler work
                        mark(f"acccopy_c{st}g{g}")
                        acc_sb = asbp.tile([65, 2, 512], F32, tag="asb",
                                           name="acc_sb")
                        for hh in range(2):
                            nc.vector.tensor_copy(acc_sb[:, hh, :], acc[hh])
                        rec = bcp.tile([1, 2, 512], F32, tag="rec",
                                       name="rec", bufs=3)
                        with nc.allow_low_precision(
                            reason="softmax reciprocal"
                        ):
                            nc.vector.reciprocal(rec, acc_sb[64:65, :, :])
                        if not last:
                            fillers.append(
                                (None, lambda a=acc_sb, r=rec, g=g, i=i:
                                 norm_finish(a, r, g, i)))
                            if g == 1:
                                for oc in range(8):
                                    item = (None,
                                            lambda st=st, oc=oc:
                                            outproj_unit(st, oc))
                                    if (idx == len(cols_order) - 2
                                            and oc >= 4):
                                        reserve.append(item[1])
                                    else:
                                        fillers.append(item)
                        else:
                            norm_finish(acc_sb, rec, g, i)

                if last:
                    for oc in range(8):
                        outproj_unit(st, oc,
                                     pool=(spsp if oc % 2 else mmps),
                                     act_copy=bool(oc % 2))

            while fillers:
                pump(1)

    nc.compile()
    return nc


def kernel(**inputs):
    global LAST_RESULTS
    from concourse.bass_utils import run_bass_kernel_spmd

    Q = np.asarray(inputs["Q"], dtype=np.float32)
    K = np.asarray(inputs["K"], dtype=np.float32)
    V = np.asarray(inputs["V"], dtype=np.float32)
    mask = np.asarray(inputs["mask"], dtype=np.float32)
    Wq = np.asarray(inputs["Wq"], dtype=np.float32)
    bq = np.asarray(inputs["bq"], dtype=np.float32)
    Wk = np.asarray(inputs["Wk"], dtype=np.float32)
    bk = np.asarray(inputs["bk"], dtype=np.float32)
    Wv = np.asarray(inputs["Wv"], dtype=np.float32)
    bv = np.asarray(inputs["bv"], dtype=np.float32)
    Wo = np.asarray(inputs["Wo"], dtype=np.float32)
    bo = np.asarray(inputs["bo"], dtype=np.float32)

    plan, dense = _analyze_mask(mask)
    key = (plan, dense.shape[0])
    if key not in _program_cache:
        _program_cache[key] = _build_program(plan, dense.shape[0])
    nc = _program_cache[key]

    import ml_dtypes
    bf16 = ml_dtypes.bfloat16
    sc = np.float32(1.0 / np.sqrt(_DK))
    xqT = [np.ascontiguousarray(Q[b].T).astype(bf16) for b in range(_B)]
    xkT = [np.ascontiguousarray(K[b].T).astype(bf16) for b in range(_B)]
    xvT = [np.ascontiguousarray(V[b].T).astype(bf16) for b in range(_B)]

    in_maps = []
    for core in range(_NCORES):
        b = core // _CPG
        rows = slice((core % _CPG) * _DPC, (core % _CPG) * _DPC + _DPC)
        in_maps.append({
            "xq": xqT[b], "xk": xkT[b], "xv": xvT[b],
            "wq": np.ascontiguousarray((Wq[rows] * sc).T).astype(bf16),
            "wk": np.ascontiguousarray(Wk[rows].T).astype(bf16),
            "wv": np.ascontiguousarray(Wv[rows].T).astype(bf16),
            "wo": np.ascontiguousarray(Wo[:, rows].T).astype(bf16),
            "bq": np.ascontiguousarray(bq[rows] * sc),
            "bk": np.ascontiguousarray(bk[rows]),
            "bvb": np.broadcast_to(bv[rows], (128, _DPC)).copy(),
            "mblk": dense,
        })

    trace = bool(int(os.environ.get("KERNEL_TRACE", "0")))
    LAST_RESULTS = run_bass_kernel_spmd(
        nc, in_maps, list(range(_NCORES)), trace=trace
    )

    out = np.empty((_B, _S, _D), np.float32)
    for b in range(_B):
        acc = np.zeros((_D, _S), np.float32)
        for c in range(_CPG):
            acc += LAST_RESULTS.results[b * _CPG + c]["y"].astype(np.float32)
        out[b] = (acc.T + bo).astype(np.float32)
    return out


# revision 39
# speedup vs baseline: 1.2366x; 1.0034x over previous
"""Multi-head attention (B=2, S=2048, D=1024, H=16) on 8 TRN2 NeuronCores.

Sharding: data-parallel over batch (2 groups of 4 cores) x head-parallel
(4 heads per core). W_q/W_k/W_v are column-sharded by head, W_o is
row-sharded; the 4 partial W_o outputs per batch are summed on the host
(the unshard step), which also undoes the device-side transposed layout.

Per-core kernel design (bf16 compute, fp32 PSUM accumulation):
  - All tensor operands (x, W_q/k/v/o, qT/kT/v, probs, attn, y) are bf16;
    PSUM accumulates in fp32, softmax denominators/normalisation in fp32.
    End-to-end relative error ~6e-3 (gate is 2e-2).
  - Host pre-transposes X and the weight slices so every matmul
    contraction sits on the partition dim; 1/sqrt(d_k) folded into W_q.
  - q/k projections produce qT/kT in [128 = 2 heads x 64 d, S] layout;
    v in natural [S, d] layout with a ones column per head so the P@V
    matmul accumulates the softmax denominator for free.
  - scores are computed transposed ([k, sq] blocks); softmax skips the
    max-subtraction (scores are O(5), exp accumulated in fp32).
  - causal structure: fully-masked [128 k x 512 sq] blocks are skipped;
    diagonal blocks are truncated to their live column range (width
    512/384/256/128) for scores, exp and P@V; the remaining triangular
    boundary is zeroed with a single shared [128,128] 0/1 mask.
  - denominator reciprocal is broadcast across partitions on the (idle)
    GpSimd engine; the per-g PSUM accumulator is copied to SBUF as soon
    as P@V finishes so the 2-bank PSUM slot frees early.
  - software-pipelined emission: each column's normalisation finish and
    output projection are deferred one column and interleaved as filler
    PE work between attention blocks (which are ACT-exp paced), so the
    PE's in-order queue never head-of-line blocks on the softmax chain.
  - all input DMAs are issued up front so the SP DMA queue never blocks
    input prefetch behind compute-dependent output stores.
"""

import os

import numpy as np

_B, _S, _D, _H, _DK = 2, 2048, 1024, 16, 64
_HPC = 4          # heads per core
_NCORES = 8
_CPG = 4          # cores per (batch) group
_DPC = _HPC * _DK # 256 projection dims per core
_NEG = -1e9

_program_cache = {}
LAST_RESULTS = None  # BassKernelResults of the most recent run (for profiling)
EMIT_LOG = []  # (instruction-id watermark, label) pairs for trace analysis


def _analyze_mask(mask):
    """Classify each [128 k, 512 sq] block of mask^T. Returns (plan, dense).

    plan[i] = tuple of (j, mode, param, off) for sq-tile i; mode 0 = no
    mask, 1 = causal-triangle boundary (masked cells live in columns
    [off, off+128) and satisfy s_rel < p), 2 = dense additive mask
    (param = index into dense blocks). ``off`` is the first live sq
    column of the block (relative to the 512-wide tile); scores/exp/PV
    are restricted to [off, 512). Fully-masked blocks are omitted.
    """
    maskT = np.ascontiguousarray(mask.T)
    plan = []
    dense = []
    p_idx = np.arange(128)[:, None]
    s_idx = np.arange(512)[None, :]
    for i in range(_S // 512):
        row = []
        for j in range(_S // 128):
            blk = maskT[j * 128:(j + 1) * 128, i * 512:(i + 1) * 512]
            nz = blk != 0.0
            if nz.all():
                continue  # fully masked: block contributes nothing
            if not nz.any():
                row.append((j, 0, 0, 0))
                continue
            base = i * 512 - j * 128
            causal = (s_idx + i * 512) < (p_idx + j * 128)
            if np.array_equal(nz, causal) and np.all(blk[nz] == 1.0):
                off = max(0, -base)
                row.append((j, 1, base, off))
            else:
                row.append((j, 2, len(dense), 0))
                dense.append(blk * np.float32(_NEG))
        if row:
            # first block must start at column 0 so the PSUM accumulator
            # is fully initialised by the start=True matmul
            j0, m0, p0, _ = row[0]
            row[0] = (j0, m0, p0, 0)
        plan.append(tuple(row))
    if dense:
        dense_np = np.stack(dense).astype(np.float32)
    else:
        dense_np = np.zeros((1, 128, 512), np.float32)
    return tuple(plan), dense_np


def _build_program(plan, nblk):
    import concourse.bass as bass  # noqa: F401  (registers engine classes)
    import concourse.tile as tile
    from concourse import bacc, mybir

    F32 = mybir.dt.float32
    BF16 = mybir.dt.bfloat16
    AF = mybir.ActivationFunctionType
    ALU = mybir.AluOpType
    ts = bass.ts

    nc = bacc.Bacc(None, target_bir_lowering=False, debug=False)

    xq = nc.dram_tensor("xq", [_D, _S], BF16, kind="ExternalInput").ap()
    xk = nc.dram_tensor("xk", [_D, _S], BF16, kind="ExternalInput").ap()
    xv = nc.dram_tensor("xv", [_D, _S], BF16, kind="ExternalInput").ap()
    wq = nc.dram_tensor("wq", [_D, _DPC], BF16, kind="ExternalInput").ap()
    wk = nc.dram_tensor("wk", [_D, _DPC], BF16, kind="ExternalInput").ap()
    wv = nc.dram_tensor("wv", [_D, _DPC], BF16, kind="ExternalInput").ap()
    wo = nc.dram_tensor("wo", [_DPC, _D], BF16, kind="ExternalInput").ap()
    bq = nc.dram_tensor("bq", [_DPC], F32, kind="ExternalInput").ap()
    bk = nc.dram_tensor("bk", [_DPC], F32, kind="ExternalInput").ap()
    bvb = nc.dram_tensor("bvb", [128, _DPC], F32, kind="ExternalInput").ap()
    mblk = nc.dram_tensor("mblk", [nblk, 128, 512], F32, kind="ExternalInput").ap()
    y = nc.dram_tensor("y", [_D, _S], BF16, kind="ExternalOutput").ap()

    has_dense = any(m == 2 for row in plan for (_, m, _, _) in row)

    EMIT_LOG.clear()

    def mark(lbl):
        EMIT_LOG.append((nc.next_id(), lbl))

    with tile.TileContext(nc) as tc:
        from contextlib import ExitStack
        with ExitStack() as ctx:
            wpool = ctx.enter_context(tc.tile_pool(name="w", bufs=1))
            cpool = ctx.enter_context(tc.tile_pool(name="const", bufs=1))
            xpool = ctx.enter_context(tc.tile_pool(name="xcol", bufs=12))
            biga = ctx.enter_context(tc.tile_pool(name="biga", bufs=1))
            probp = ctx.enter_context(tc.tile_pool(name="probs", bufs=7))
            asbp = ctx.enter_context(tc.tile_pool(name="asb", bufs=4))
            bcp = ctx.enter_context(tc.tile_pool(name="bc", bufs=2))
            yp = ctx.enter_context(tc.tile_pool(name="y", bufs=4))
            mpool = (
                ctx.enter_context(tc.tile_pool(name="mstream", bufs=2))
                if has_dense else None
            )
            mmps = ctx.enter_context(tc.tile_pool(name="mmps", bufs=2, space="PSUM"))
            spsp = ctx.enter_context(tc.tile_pool(name="sps", bufs=2, space="PSUM"))
            accp = ctx.enter_context(tc.tile_pool(name="acc", bufs=2, space="PSUM"))

            xq_r = xq.rearrange("(m p) s -> p m s", p=128)
            xk_r = xk.rearrange("(m p) s -> p m s", p=128)
            xv_r = xv.rearrange("(m p) s -> p m s", p=128)
            wq_r = wq.rearrange("(m p) d -> p m d", p=128)
            wk_r = wk.rearrange("(m p) d -> p m d", p=128)
            wv_r = wv.rearrange("(m p) d -> p m d", p=128)

            # --- all input DMAs issued up front; column 0's q tensors,
            # then the (tiny) biases, then k/v, then the rest.
            bq_sb = cpool.tile([128, 2], F32, tag="bq")
            bk_sb = cpool.tile([128, 2], F32, tag="bk")
            wq_sb = wpool.tile([128, 8, _DPC], BF16, tag="wq")
            wk_sb = wpool.tile([128, 8, _DPC], BF16, tag="wk")
            wv_sb = wpool.tile([128, 8, _DPC], BF16, tag="wv")
            xq_t = [xpool.tile([128, 8, 512], BF16, tag="xcol",
                               name=f"xq_t{c}") for c in range(4)]
            xk_t = [xpool.tile([128, 8, 512], BF16, tag="xcol",
                               name=f"xk_t{c}") for c in range(4)]
            xv_t = [xpool.tile([128, 8, 512], BF16, tag="xcol",
                               name=f"xv_t{c}") for c in range(4)]
            bvb_sb = cpool.tile([128, _DPC], F32, tag="bvb")
            for w_sb, w_r, x_t, x_r in (
                (wq_sb, wq_r, xq_t[0], xq_r),
                (wk_sb, wk_r, xk_t[0], xk_r),
                (wv_sb, wv_r, xv_t[0], xv_r),
            ):
                for lo, hi in ((0, 4), (4, 8)):
                    nc.sync.dma_start(out=w_sb[:, lo:hi, :],
                                      in_=w_r[:, lo:hi, :])
                    nc.sync.dma_start(out=x_t[:, lo:hi, :],
                                      in_=x_r[:, lo:hi, ts(0, 512)])
                if w_sb is wq_sb:
                    nc.sync.dma_start(
                        out=bq_sb, in_=bq.rearrange("(h p) -> p h", p=128))
                    nc.sync.dma_start(
                        out=bk_sb, in_=bk.rearrange("(h p) -> p h", p=128))
                if w_sb is wv_sb:
                    nc.sync.dma_start(out=bvb_sb, in_=bvb)

            wo_sb = wpool.tile([128, 2, _D], BF16, tag="wo")

            # remaining x columns (prefetch, in processing order); wo is
            # only needed at the first out-projection (~45us)
            for c in (1, 3, 2):
                nc.sync.dma_start(out=xq_t[c], in_=xq_r[:, :, ts(c, 512)])
                nc.sync.dma_start(out=xk_t[c], in_=xk_r[:, :, ts(c, 512)])
                nc.sync.dma_start(out=xv_t[c], in_=xv_r[:, :, ts(c, 512)])
                if c == 1:
                    nc.sync.dma_start(
                        out=wo_sb, in_=wo.rearrange("(c p) o -> p c o", p=128))

            resident_mask = has_dense and nblk <= 2
            if resident_mask:
                mask_sb = cpool.tile([128, nblk, 512], F32, tag="mask")
                nc.sync.dma_start(
                    out=mask_sb, in_=mblk.rearrange("n p s -> p n s")
                )

            # shared triangular boundary mask: keep cell (p, s_rel) iff
            # s_rel >= p (after live-range shift every causal diagonal
            # block reduces to this)
            use_m01 = any(m == 1 for row in plan for (_, m, _, _) in row)
            if use_m01:
                m01 = cpool.tile([128, 128], BF16, tag="m01")
                nc.vector.memset(m01, 1.0)
                nc.gpsimd.affine_select(
                    out=m01, in_=m01,
                    compare_op=ALU.is_ge, fill=0.0, base=0,
                    channel_multiplier=-1, pattern=[[1, 128]],
                )

            # --- big SBUF state ---
            qT = biga.tile([128, 2, _S], BF16, tag="qT")
            kT = biga.tile([128, 2, _S], BF16, tag="kT")
            vsb = biga.tile([128, 16, _HPC * 65], BF16, tag="v")
            attn = biga.tile([128, 2, _S], BF16, tag="attn")

            # ones columns of v (softmax denominator trick): one strided
            # memset over all 16 x 4 ones columns
            vsb_ones = vsb.rearrange("p s (h x) -> p s h x", x=65)[:, :, :, 64:65]
            nc.vector.memset(vsb_ones, 1.0)

            # v-projection emitted lazily per 512-wide k-column group, the
            # first time any PV needs a chunk from it
            v_pending = set(range(4))

            def ensure_vgroup(col):
                if col not in v_pending:
                    return
                v_pending.discard(col)
                mark(f"vproj{col}")
                for c in range(4):
                    vps = mmps.tile([128, 512], F32, tag="mm", name="vps")
                    for m in range(8):
                        nc.tensor.matmul(
                            vps[:, 0:_DPC], lhsT=xv_t[col][:, m, ts(c, 128)],
                            rhs=wv_sb[:, m, :], start=(m == 0), stop=(m == 7),
                        )
                    sc = col * 4 + c
                    nc.vector.tensor_add(
                        vsb[:, sc, 0:260].rearrange(
                            "p (h x) -> p h x", x=65)[:, :, 0:64],
                        vps[:, 0:_DPC].rearrange("p (h x) -> p h x", x=64),
                        bvb_sb.rearrange("p (h x) -> p h x", x=64),
                    )

            def proj_unit(which, st, dh):
                # one q- or k-projection unit: 8 accumulating matmuls
                # (contraction over D) + DVE bias add into qT/kT
                mark(f"proj_{which}{st}d{dh}")
                x_t, w_sb, b_sb, dst = (
                    (xq_t[st], wq_sb, bq_sb, qT) if which == "q"
                    else (xk_t[st], wk_sb, bk_sb, kT)
                )
                ps = mmps.tile([128, 512], F32, tag="mm", name=f"{which}ps")
                for m in range(8):
                    nc.tensor.matmul(
                        ps, lhsT=w_sb[:, m, ts(dh, 128)], rhs=x_t[:, m, :],
                        start=(m == 0), stop=(m == 7),
                    )
                nc.vector.tensor_scalar(
                    dst[:, dh, ts(st, 512)], ps, b_sb[:, dh:dh + 1], None,
                    ALU.add,
                )

            # deferred work units (FIFO), pumped between attention blocks
            # so the PE always has ready work while exp latency drains.
            # ``reserve`` units are held for the very end of the kernel to
            # fill the PE while the last column's softmax chain drains.
            fillers = []   # entries: (proj_col_or_None, emit_fn)
            reserve = []

            def pump(n=1):
                for _ in range(min(n, len(fillers))):
                    fillers.pop(0)[1]()

            def flush_proj_upto(col):
                # correctness: Tile tracks dependencies in emission order,
                # so any projection for a column whose kT/qT this column's
                # attention reads must be emitted before the blocks
                keep = []
                for ent in fillers:
                    if ent[0] is not None and ent[0] <= col:
                        ent[1]()
                    else:
                        keep.append(ent)
                fillers[:] = keep

            def norm_finish(acc_sb, rec, g, i):
                # broadcast the reciprocal across partitions on GpSimd,
                # then normalise into attn (all-SBUF DVE muls)
                mark(f"normfin_g{g}_c{i}")
                bc_sb = bcp.tile([64, 2, 512], F32, tag="bc", name="bc_sb")
                nc.gpsimd.partition_broadcast(bc_sb, rec, channels=64)
                for hh in range(2):
                    nc.vector.tensor_mul(
                        attn[hh * 64:(hh + 1) * 64, g, ts(i, 512)],
                        acc_sb[0:64, hh, :], bc_sb[:, hh, :],
                    )

            def fast_norm_panel(acc, g, i, lo, hi):
                # per-hh reciprocal/broadcast/multiply for sq columns
                # [lo, hi) straight off the PSUM accumulator
                w = hi - lo
                recl = [None, None]
                for hh in range(2):
                    recl[hh] = bcp.tile([1, 512], F32, tag="recl",
                                        name="recl", bufs=2)
                    with nc.allow_low_precision(
                        reason="softmax reciprocal"
                    ):
                        nc.vector.reciprocal(
                            recl[hh][:, 0:w], acc[hh][64:65, lo:hi])
                bcl = [None, None]
                for hh in range(2):
                    bcl[hh] = bcp.tile([64, 512], F32, tag="bcl",
                                       name="bcl", bufs=2)
                    nc.gpsimd.partition_broadcast(
                        bcl[hh][:, 0:w], recl[hh][:, 0:w], channels=64)
                for hh in range(2):
                    nc.vector.tensor_mul(
                        attn[hh * 64:(hh + 1) * 64, g,
                             i * 512 + lo:i * 512 + hi],
                        acc[hh][0:64, lo:hi], bcl[hh][:, 0:w],
                    )

            def outproj_unit_last(st, oc, pool, act_copy):
                # final-column out-projection, split by sq panel into two
                # independent half-bank PSUM tiles so each half's matmuls,
                # PSUM->SBUF copy and store start as soon as its slice of
                # attn is normalised
                mark(f"outproj{st}_oc{oc}")
                tag = "mm" if pool is mmps else "sps"
                y_sb = yp.tile([128, 512], BF16, tag="y", name="y_sb")
                for lo, hi in ((0, 256), (256, 512)):
                    yps = pool.tile([128, 256], F32, tag=tag, name="yps")
                    for cc in range(2):
                        nc.tensor.matmul(
                            yps, lhsT=wo_sb[:, cc, ts(oc, 128)],
                            rhs=attn[:, cc, st * 512 + lo:st * 512 + hi],
                            start=(cc == 0), stop=(cc == 1),
                        )
                    if act_copy:
                        nc.scalar.copy(y_sb[:, lo:hi], yps)
                    else:
                        nc.vector.tensor_copy(y_sb[:, lo:hi], yps)
                nc.sync.dma_start(
                    out=y[oc * 128:(oc + 1) * 128, ts(st, 512)], in_=y_sb
                )

            def outproj_unit(st, oc, pool=None, act_copy=False,
                             pool_dma=False):
                # output projection for one 128-row slice of y (partial)
                mark(f"outproj{st}_oc{oc}")
                pool = pool or mmps
                tag = "mm" if pool is mmps else "sps"
                yps = pool.tile([128, 512], F32, tag=tag, name="yps")
                for cc in range(2):
                    nc.tensor.matmul(
                        yps, lhsT=wo_sb[:, cc, ts(oc, 128)],
                        rhs=attn[:, cc, ts(st, 512)],
                        start=(cc == 0), stop=(cc == 1),
                    )
                y_sb = yp.tile([128, 512], BF16, tag="y", name="y_sb")
                if act_copy:
                    nc.scalar.copy(y_sb, yps)
                else:
                    nc.vector.tensor_copy(y_sb, yps)
                eng = nc.gpsimd if pool_dma else nc.sync
                eng.dma_start(
                    out=y[oc * 128:(oc + 1) * 128, ts(st, 512)], in_=y_sb
                )

            # --- fused pipeline over sq columns. Processing order puts
            # the largest column (3) third so the deferred work of its
            # predecessor fills its exp-paced bubbles, and column 2 last
            # (its own bubbles absorb column 3's deferred norm/outproj).
            cols_order = (0, 1, 3, 2)
            for idx, st in enumerate(cols_order):
                nxt = cols_order[idx + 1] if idx + 1 < len(cols_order) else None
                last = nxt is None
                if idx == 0:
                    # column 0's q projection, both dh slots interleaved at
                    # half-contraction granularity: dh1's first m-chunks
                    # only need the first xq half-DMA, so they fill the PE
                    # while the second half is still in flight
                    mark("proj_q0_split")
                    qps2 = [mmps.tile([128, 512], F32, tag="mm",
                                      name=f"qps{d}") for d in range(2)]
                    for dh in range(2):
                        for m in range(4):
                            nc.tensor.matmul(
                                qps2[dh], lhsT=wq_sb[:, m, ts(dh, 128)],
                                rhs=xq_t[0][:, m, :],
                                start=(m == 0), stop=False,
                            )
                    for dh in range(2):
                        for m in range(4, 8):
                            nc.tensor.matmul(
                                qps2[dh], lhsT=wq_sb[:, m, ts(dh, 128)],
                                rhs=xq_t[0][:, m, :],
                                start=False, stop=(m == 7),
                            )
                        nc.vector.tensor_scalar(
                            qT[:, dh, ts(0, 512)], qps2[dh],
                            bq_sb[:, dh:dh + 1], None, ALU.add,
                        )
                    proj_unit("k", st, 0)
                    fillers.append((0, lambda: proj_unit("k", 0, 1)))
                # queue all remaining columns' projections as filler work
                if idx == 0:
                    for qc in cols_order[1:]:
                        for dh in range(2):
                            fillers.append(
                                (qc,
                                 lambda dh=dh, c=qc: proj_unit("q", c, dh)))
                            fillers.append(
                                (qc,
                                 lambda dh=dh, c=qc: proj_unit("k", c, dh)))
                # correctness: every kT column this column's blocks read
                # must be projected in emission order first
                need = max((b[0] // 4 for b in plan[st]), default=0)
                flush_proj_upto(max(st, need))

                i = st
                blocks = plan[i]
                nj = len(blocks)
                for g in range(2):
                    # two independent 1-bank accumulators (one per hh) so
                    # the next group's P@V unblocks per-half as each copy
                    # drains, instead of waiting for the full 2-bank tile
                    acc = [accp.tile([65, 512], F32, tag="acc",
                                     name=f"acc{st}{g}h{hh}")
                           for hh in range(2)]

                    def emit_pv(pend, start, stop):
                        j_p, off_p, probs_p = pend
                        for hh in range(2):
                            h = 2 * g + hh
                            nc.tensor.matmul(
                                acc[hh][:, off_p:512],
                                lhsT=vsb[:, j_p, h * 65:(h + 1) * 65],
                                rhs=probs_p[:, hh, off_p:512],
                                start=start, stop=stop,
                            )

                    tail_g = last and g == 1
                    if tail_g:
                        # flush deferred work now so its DVE traffic lands
                        # ahead of the softmax-chain ops in the DVE queue
                        while fillers:
                            pump(1)
                        for r in reserve:
                            r()
                    pend = None  # scores/exp run one block ahead of PV
                    for bi, (j, mode, param, off) in enumerate(blocks):
                        mark(f"blk_c{st}g{g}j{j}")
                        ensure_vgroup(j // 4)
                        sps = spsp.tile([128, 2, 512], F32, tag="sps", name="sps")
                        for hh in range(2):
                            nc.tensor.matmul(
                                sps[:, hh, off:512],
                                lhsT=kT[hh * 64:(hh + 1) * 64, g, ts(j, 128)],
                                rhs=qT[hh * 64:(hh + 1) * 64, g,
                                       i * 512 + off:(i + 1) * 512],
                                start=True, stop=True,
                            )
                        if mode == 2:
                            if resident_mask:
                                mt = mask_sb[:, param, :]
                            else:
                                mt = mpool.tile([128, 512], F32, tag="mtile",
                                                name="mt")
                                nc.sync.dma_start(out=mt, in_=mblk[param])
                            for hh in range(2):
                                nc.vector.tensor_add(
                                    sps[:, hh, :], sps[:, hh, :], mt
                                )
                        probs = probp.tile([128, 2, 512], BF16, tag="probs",
                                           name="probs")
                        nc.scalar.activation(
                            probs[:, :, off:512], sps[:, :, off:512], AF.Exp
                        )
                        if mode == 1:
                            # masked cells sit in columns [off, off+128):
                            # s_rel < p relative to the live window
                            for hh in range(2):
                                nc.vector.tensor_mul(
                                    probs[:, hh, off:off + 128],
                                    probs[:, hh, off:off + 128],
                                    m01,
                                )
                        if pend is not None:
                            emit_pv(pend, start=(bi == 1), stop=False)
                            pump(1)
                        pend = (j, off, probs)
                    emit_pv(pend, start=(nj == 1), stop=True)
                    if tail_g:
                        fast_norm_panel(acc, g, i, 0, 512)
                    pump(3)

                    if not tail_g:
                        # copy the accumulator out of PSUM promptly (frees
                        # the 2-bank slot for the other head-pair group) and
                        # take the reciprocal; the rest of the normalisation
                        # is deferred as filler work
                        mark(f"acccopy_c{st}g{g}")
                        acc_sb = asbp.tile([65, 2, 512], F32, tag="asb",
                                           name="acc_sb")
                        for hh in range(2):
                            nc.vector.tensor_copy(acc_sb[:, hh, :], acc[hh])
                        rec = bcp.tile([1, 2, 512], F32, tag="rec",
                                       name="rec", bufs=3)
                        with nc.allow_low_precision(
                            reason="softmax reciprocal"
                        ):
                            nc.vector.reciprocal(rec, acc_sb[64:65, :, :])
                        if not last:
                            fillers.append(
                                (None, lambda a=acc_sb, r=rec, g=g, i=i:
                                 norm_finish(a, r, g, i)))
                            if g == 1:
                                for oc in range(8):
                                    item = (None,
                                            lambda st=st, oc=oc:
                                            outproj_unit(st, oc))
                                    if ((idx in (0, len(cols_order) - 2)
                                         and oc >= 4)
                                            or (idx == 1 and oc >= 6)):
                                        # col 1 has filler excess; hold
                                        # half of col 0's and col 3's
                                        # outproj for the last column's
                                        # dry stretches
                                        reserve.append(item[1])
                                    else:
                                        fillers.append(item)
                        else:
                            norm_finish(acc_sb, rec, g, i)

                if last:
                    for oc in range(8):
                        outproj_unit(st, oc,
                                     pool=(spsp if oc % 2 else mmps),
                                     act_copy=bool(oc % 2))

            while fillers:
                pump(1)

    nc.compile()
    return nc


def kernel(**inputs):
    global LAST_RESULTS
    from concourse.bass_utils import run_bass_kernel_spmd

    Q = np.asarray(inputs["Q"], dtype=np.float32)
    K = np.asarray(inputs["K"], dtype=np.float32)
    V = np.asarray(inputs["V"], dtype=np.float32)
    mask = np.asarray(inputs["mask"], dtype=np.float32)
    Wq = np.asarray(inputs["Wq"], dtype=np.float32)
    bq = np.asarray(inputs["bq"], dtype=np.float32)
    Wk = np.asarray(inputs["Wk"], dtype=np.float32)
    bk = np.asarray(inputs["bk"], dtype=np.float32)
    Wv = np.asarray(inputs["Wv"], dtype=np.float32)
    bv = np.asarray(inputs["bv"], dtype=np.float32)
    Wo = np.asarray(inputs["Wo"], dtype=np.float32)
    bo = np.asarray(inputs["bo"], dtype=np.float32)

    plan, dense = _analyze_mask(mask)
    key = (plan, dense.shape[0])
    if key not in _program_cache:
        _program_cache[key] = _build_program(plan, dense.shape[0])
    nc = _program_cache[key]

    import ml_dtypes
    bf16 = ml_dtypes.bfloat16
    sc = np.float32(1.0 / np.sqrt(_DK))
    xqT = [np.ascontiguousarray(Q[b].T).astype(bf16) for b in range(_B)]
    xkT = [np.ascontiguousarray(K[b].T).astype(bf16) for b in range(_B)]
    xvT = [np.ascontiguousarray(V[b].T).astype(bf16) for b in range(_B)]

    in_maps = []
    for core in range(_NCORES):
        b = core // _CPG
        rows = slice((core % _CPG) * _DPC, (core % _CPG) * _DPC + _DPC)
        in_maps.append({
            "xq": xqT[b], "xk": xkT[b], "xv": xvT[b],
            "wq": np.ascontiguousarray((Wq[rows] * sc).T).astype(bf16),
            "wk": np.ascontiguousarray(Wk[rows].T).astype(bf16),
            "wv": np.ascontiguousarray(Wv[rows].T).astype(bf16),
            "wo": np.ascontiguousarray(Wo[:, rows].T).astype(bf16),
            "bq": np.ascontiguousarray(bq[rows] * sc),
            "bk": np.ascontiguousarray(bk[rows]),
            "bvb": np.broadcast_to(bv[rows], (128, _DPC)).copy(),
            "mblk": dense,
        })

    trace = bool(int(os.environ.get("KERNEL_TRACE", "0")))
    LAST_RESULTS = run_bass_kernel_spmd(
        nc, in_maps, list(range(_NCORES)), trace=trace
    )

    out = np.empty((_B, _S, _D), np.float32)
    for b in range(_B):
        acc = np.zeros((_D, _S), np.float32)
        for c in range(_CPG):
            acc += LAST_RESULTS.results[b * _CPG + c]["y"].astype(np.float32)
        out[b] = (acc.T + bo).astype(np.float32)
    return out
